# revision 1
# baseline (speedup 1.0000x reference)
"""Bass/Trainium2 kernel for the 2-branch GCN (gnn_message_passing).

Computation (reference):
    per branch i in {a, b}:
        u_i = x_i @ W1_i                                  [N, H]
        h_i = relu(spmm(A, u_i) + b1_i)                   [N, H]
        v_i = h_i @ W2_i                                  [N, H]
        g_i = spmm(A, v_i) + b2_i                         [N, H]
        z_i = log_softmax(g_i @ LW_i + Lb_i)              [N, H]
    out = log_softmax(concat(z_a, z_b) @ LW + Lb)         [N, C]
where spmm(A, u)[d] = sum_{e: dst[e]=d} w[e] * u[src[e]].

Strategy (8 NeuronCores, node-sharded):
  - Core c owns node rows [c*S, (c+1)*S), S = N/8.
  - Dense matmuls on PE in bf16 (fp32 PSUM accumulate); activations
    feature-major (transposed) so they feed the next matmul's lhsT.
  - SpMM: AllGather the (concat-branch) activation table [N, 2H] bf16,
    then per 128-edge chunk: indirect-DMA row gather + one PE matmul
    against a host-precomputed one-hot aggregation matrix M
    (M[e, dst_local] = w[e]), accumulating per-dst-tile in PSUM.
  - Both branches share each gather (concat features -> 1KB rows).
  - Edge chunks are identical for both spmm layers (same graph) so the
    M / index tensors are built once on the host and reused.
"""

import sys

if "/opt/trn_rl_repo" not in sys.path:
    sys.path.insert(0, "/opt/trn_rl_repo")

import numpy as np
import ml_dtypes

import concourse.bass as bass
import concourse.bacc as bacc
import concourse.mybir as mybir
import concourse.tile as tile
from concourse.tile import TileContext
from concourse.masks import make_identity
from concourse.bass_utils import run_bass_kernel_spmd

import contextlib
import concourse.hw_specs as _hw_specs
import concourse.bacc as _bacc_mod


@contextlib.contextmanager
def _pinned_act_tables():
    """During compile, make every activation-function table except the
    all-purpose one look empty so bacc's table-load inserter picks a single
    table for the whole program (one LoadActFuncSet instead of ~300).
    Table ids/order are unchanged; restored afterwards."""
    orig = _bacc_mod.get_activation_tables

    def pinned(arch):
        tabs = orig(arch)
        keep = "natural_log_exp_and_others"
        if keep in tabs:
            tabs = {k: (v if k == keep else set()) for k, v in tabs.items()}
        return tabs

    _bacc_mod.get_activation_tables = pinned
    try:
        yield
    finally:
        _bacc_mod.get_activation_tables = orig

BF16 = ml_dtypes.bfloat16
dt = mybir.dt
P = 128
N_CORES = 8
TBL_DT = dt.bfloat16          # layer-1 gather-table dtype (u)
TBL2_DT = dt.bfloat16         # layer-2 gather-table dtype (v)


# ----------------------------------------------------------------------------
# Host-side edge preprocessing
# ----------------------------------------------------------------------------

def preprocess_edges(edge_src, edge_dst, edge_w, N, S):
    """Chunk edges per (dst-core, 128-dst tile, 32-dst window, src-half).

    M is banded: each 128-edge chunk's aggregation matrix is [128, 32]
    (one 32-dst window). Chunk counts are padded to a uniform shape across
    cores. Within a tile, chunks are numbered lo-half window-major first,
    then hi-half window-major.

    Returns (cpw_lo, cpw_hi, M_list, idxl_list, idxh_list).
    """
    edge_src = np.asarray(edge_src).astype(np.int64)
    edge_dst = np.asarray(edge_dst).astype(np.int64)
    edge_w = np.asarray(edge_w, dtype=np.float32)
    n_tiles = (S + P - 1) // P
    n_win = (S + 31) // 32
    HALF = N // 2

    per_core = []
    cnt = np.zeros((N_CORES, 2 * n_win), dtype=np.int64)   # group = win*2+half
    for c in range(N_CORES):
        sel = (edge_dst >= c * S) & (edge_dst < (c + 1) * S)
        dl = edge_dst[sel] - c * S
        sg = edge_src[sel]
        w = edge_w[sel]
        hi = (sg >= HALF).astype(np.int64)
        order = np.lexsort((dl, hi, dl >> 5))
        dl, sg, w, hi = dl[order], sg[order], w[order], hi[order]
        t = dl >> 7
        win = dl >> 5
        gid = win * 2 + hi
        cnt[c] = np.bincount(gid, minlength=2 * n_win)
        per_core.append((dl, sg, w, hi, t, win, gid))

    cg = (cnt.max(axis=0) + P - 1) // P
    cpw_lo = np.maximum(1, cg[0::2]).astype(np.int64)   # [n_win]
    cpw_hi = cg[1::2].astype(np.int64)                  # [n_win]

    # chunk numbering: per tile, lo chunks of its windows, then hi chunks
    wins_of_tile = [list(range(4 * t, min(4 * t + 4, n_win)))
                    for t in range(n_tiles)]
    chunk_of_group = np.zeros(2 * n_win, dtype=np.int64)  # first chunk per group
    nxt = 0
    for t in range(n_tiles):
        for wg in wins_of_tile[t]:
            chunk_of_group[2 * wg] = nxt
            nxt += cpw_lo[wg]
        for wg in wins_of_tile[t]:
            chunk_of_group[2 * wg + 1] = nxt
            nxt += cpw_hi[wg]
    nchunk = nxt
    # lo/hi chunk numbering for gather-index columns (per half, tile-major)
    clo_base = np.concatenate([[0], np.cumsum(cpw_lo)])
    chi_base = np.concatenate([[0], np.cumsum(cpw_hi)])
    CLO, CHI = int(clo_base[-1]), int(chi_base[-1])

    M_list, idxl_list, idxh_list = [], [], []
    for c in range(N_CORES):
        dl, sg, w, hi, t, win, gid = per_core[c]
        gcnt = np.bincount(gid, minlength=2 * n_win)
        gstart = np.concatenate([[0], np.cumsum(gcnt)])
        pos = np.arange(len(dl)) - gstart[gid]
        ci = pos >> 7
        lane = (pos & 127).astype(np.int64)
        chunk = chunk_of_group[gid] + ci
        M = np.zeros((P, nchunk * 32), dtype=BF16)
        M[lane, chunk * 32 + (dl & 31)] = w.astype(BF16)

        # gather token = (chunk index within the (tile,half) call)*128 + lane.
        # Within a call, chunks are the half's window-major chunks of the
        # tile; their per-half numbering (clo/chi) is already call-contiguous.
        idxl = np.zeros((P, CLO * 8), dtype=np.int16)
        idxh = np.zeros((P, max(CHI, 1) * 8), dtype=np.int16)
        halfchunk = np.where(hi == 0, clo_base[win] + ci, chi_base[win] + ci)
        tile_first = np.where(hi == 0,
                              clo_base[4 * t],
                              chi_base[np.minimum(4 * t, n_win - 1)])
        j = (halfchunk - tile_first) * P + lane
        for (arr, msk, off, base) in ((idxl, hi == 0, 0, clo_base),
                                      (idxh, hi == 1, HALF, chi_base)):
            jj = j[msk]
            col = base[4 * t[msk]] * 8 + (jj >> 4)
            row = (jj & 15)
            val = (sg[msk] - off).astype(np.int16)
            for g in range(8):
                arr[16 * g + row, col] = val
        M_list.append(M)
        idxl_list.append(idxl)
        idxh_list.append(idxh)
    return (cpw_lo, cpw_hi, M_list, idxl_list, idxh_list)


# ----------------------------------------------------------------------------
# Bass program
# ----------------------------------------------------------------------------

def build_nc(N, F0, H, C, S, cpw_lo, cpw_hi, single_core=False, rep=1):
    n_tiles = (S + P - 1) // P
    n_win = (S + 31) // 32
    HALF = N // 2
    clo_base = np.concatenate([[0], np.cumsum(cpw_lo)])
    chi_base = np.concatenate([[0], np.cumsum(cpw_hi)])
    CLO, CHI = int(clo_base[-1]), int(chi_base[-1])
    nchunk = CLO + CHI
    wins_of_tile = [list(range(4 * t, min(4 * t + 4, n_win)))
                    for t in range(n_tiles)]
    KF = F0 // P       # k-chunks of F0 (4)
    KH = H // P        # k-chunks of H (2)
    H2 = 2 * H

    nc = bacc.Bacc("TRN2", num_devices=1 if single_core else N_CORES,
                   dynamic_dma_scratch_size=24576)

    # --- I/O ---
    x0T = nc.declare_dram_parameter("x0T", [F0, S], dt.bfloat16, isOutput=False)
    x1T = nc.declare_dram_parameter("x1T", [F0, S], dt.bfloat16, isOutput=False)
    W1a = nc.declare_dram_parameter("W1a", [F0, H], dt.bfloat16, isOutput=False)
    W1b = nc.declare_dram_parameter("W1b", [F0, H], dt.bfloat16, isOutput=False)
    W2a = nc.declare_dram_parameter("W2a", [H, H], dt.bfloat16, isOutput=False)
    W2b = nc.declare_dram_parameter("W2b", [H, H], dt.bfloat16, isOutput=False)
    LWa = nc.declare_dram_parameter("LWa", [H, H], dt.bfloat16, isOutput=False)
    LWb = nc.declare_dram_parameter("LWb", [H, H], dt.bfloat16, isOutput=False)
    LWf = nc.declare_dram_parameter("LWf", [H2, C], dt.bfloat16, isOutput=False)
    b1 = nc.declare_dram_parameter("b1", [P, H2], dt.bfloat16, isOutput=False)
    b2 = nc.declare_dram_parameter("b2", [P, H2], dt.bfloat16, isOutput=False)
    lba = nc.declare_dram_parameter("lba", [P, H], dt.bfloat16, isOutput=False)
    lbb = nc.declare_dram_parameter("lbb", [P, H], dt.bfloat16, isOutput=False)
    lbf = nc.declare_dram_parameter("lbf", [P, C], dt.bfloat16, isOutput=False)
    Mt = nc.declare_dram_parameter("M", [P, nchunk * 32], dt.bfloat16, isOutput=False)
    IDXL = nc.declare_dram_parameter("IDXL", [P, CLO * 8], dt.int16, isOutput=False)
    IDXH = nc.declare_dram_parameter("IDXH", [P, max(CHI, 1) * 8], dt.int16, isOutput=False)
    out_t = nc.declare_dram_parameter("out", [S, C], dt.float32, isOutput=True)

    # --- internal DRAM ---
    u_loc = nc.dram_tensor("u_loc", [S, H2], TBL_DT)
    v_loc = nc.dram_tensor("v_loc", [S, H2], TBL2_DT)
    if single_core:
        U = nc.declare_dram_parameter("Uin", [N, H2], TBL_DT, isOutput=False)
        V = nc.declare_dram_parameter("Vin", [N, H2], TBL2_DT, isOutput=False)
    else:
        U = nc.dram_tensor("U", [N, H2], TBL_DT, addr_space="Shared")
        V = nc.dram_tensor("V", [N, H2], TBL2_DT, addr_space="Shared")
    groups = [list(range(N_CORES))]

    with TileContext(nc, num_cores=N_CORES) as tc:
        import contextlib
        ctx = contextlib.ExitStack()
        with ctx:
            perm = ctx.enter_context(tc.tile_pool(name="perm", bufs=1))
            big = ctx.enter_context(tc.tile_pool(name="big", bufs=1))
            mpool = ctx.enter_context(tc.tile_pool(name="mpool", bufs=2))
            msgp = ctx.enter_context(tc.tile_pool(name="msgp", bufs=2))
            sb = ctx.enter_context(tc.tile_pool(name="sb", bufs=2))
            stat = ctx.enter_context(tc.tile_pool(name="stat", bufs=4))
            ps_big = ctx.enter_context(tc.tile_pool(name="ps_big", bufs=2, space="PSUM"))
            ps_d = ctx.enter_context(tc.tile_pool(name="ps_d", bufs=2, space="PSUM"))
            ps_t = ctx.enter_context(tc.tile_pool(name="ps_t", bufs=2, space="PSUM"))
            ps_f = ctx.enter_context(tc.tile_pool(name="ps_f", bufs=2, space="PSUM"))

            # persistent small tiles
            ident = perm.tile([P, P], dt.bfloat16, tag="ident")
            make_identity(nc, ident[:])
            w1a_t = [perm.tile([P, H], dt.bfloat16, name=f"w1a{k}", tag=f"w1a{k}") for k in range(KF)]
            w1b_t = [perm.tile([P, H], dt.bfloat16, name=f"w1b{k}", tag=f"w1b{k}") for k in range(KF)]
            w2a_t = [perm.tile([P, H], dt.bfloat16, name=f"w2a{k}", tag=f"w2a{k}") for k in range(KH)]
            w2b_t = [perm.tile([P, H], dt.bfloat16, name=f"w2b{k}", tag=f"w2b{k}") for k in range(KH)]
            lwa_t = [perm.tile([P, H], dt.bfloat16, name=f"lwa{k}", tag=f"lwa{k}") for k in range(KH)]
            lwb_t = [perm.tile([P, H], dt.bfloat16, name=f"lwb{k}", tag=f"lwb{k}") for k in range(KH)]
            lwf_t = [perm.tile([P, C], dt.bfloat16, name=f"lwf{k}", tag=f"lwf{k}") for k in range(2 * KH)]
            for k in range(KF):
                nc.sync.dma_start(out=w1a_t[k][:], in_=W1a[k * P:(k + 1) * P, :])
                nc.sync.dma_start(out=w1b_t[k][:], in_=W1b[k * P:(k + 1) * P, :])
            for k in range(KH):
                nc.sync.dma_start(out=w2a_t[k][:], in_=W2a[k * P:(k + 1) * P, :])
                nc.sync.dma_start(out=w2b_t[k][:], in_=W2b[k * P:(k + 1) * P, :])
                nc.sync.dma_start(out=lwa_t[k][:], in_=LWa[k * P:(k + 1) * P, :])
                nc.sync.dma_start(out=lwb_t[k][:], in_=LWb[k * P:(k + 1) * P, :])
            for k in range(2 * KH):
                nc.sync.dma_start(out=lwf_t[k][:], in_=LWf[k * P:(k + 1) * P, :])
            b1_t = perm.tile([P, H2], dt.bfloat16, tag="b1")
            b2_t = perm.tile([P, H2], dt.bfloat16, tag="b2")
            lba_t = perm.tile([P, H], dt.bfloat16, tag="lba")
            lbb_t = perm.tile([P, H], dt.bfloat16, tag="lbb")
            lbf_t = perm.tile([P, C], dt.bfloat16, tag="lbf")
            nc.sync.dma_start(out=b1_t[:], in_=b1[:])
            nc.sync.dma_start(out=b2_t[:], in_=b2[:])
            nc.sync.dma_start(out=lba_t[:], in_=lba[:])
            nc.sync.dma_start(out=lbb_t[:], in_=lbb[:])
            nc.sync.dma_start(out=lbf_t[:], in_=lbf[:])
            idxl_t = perm.tile([P, CLO * 8], dt.int16, tag="idxl")
            nc.sync.dma_start(out=idxl_t[:], in_=IDXL[:])
            idxh_t = perm.tile([P, max(CHI, 1) * 8], dt.int16, tag="idxh")
            nc.sync.dma_start(out=idxh_t[:], in_=IDXH[:])

            # 8 big feature-major tiles [P, S]; reused across phases:
            #   phase A: bigT[0:KF] = x0T, bigT[KF:2KF] = x1T
            #   phase C out: bigT[0:2KH*?]... haT = bigT[0:KH], hbT = bigT[KH:2KH]
            #   phase F out: gaT = bigT[2KH:3KH], gbT = bigT[3KH:4KH]
            #   phase G out: zT = bigT[0:2KH] (a|b concat features)
            n_big = max(2 * KF, 4 * KH)
            bigT = [big.tile([P, S], dt.bfloat16, name=f"bigT{i}", tag=f"bigT{i}") for i in range(n_big)]
            for k in range(KF):
                nc.sync.dma_start(out=bigT[k][:], in_=x0T[k * P:(k + 1) * P, :])
                nc.sync.dma_start(out=bigT[KF + k][:], in_=x1T[k * P:(k + 1) * P, :])

            def mtile(m):
                ms = m * P
                return ms, min(P, S - ms)

            # ---------------- Phase A: u = x @ W1 (both branches) ----------
            for m in range(n_tiles):
                ms, mw = mtile(m)
                pa = ps_d.tile([P, H], dt.float32, tag="ps_d")
                pb = ps_d.tile([P, H], dt.float32, tag="ps_d")
                for k in range(KF):
                    nc.tensor.matmul(pa[:mw, :], lhsT=bigT[k][:, ms:ms + mw],
                                     rhs=w1a_t[k][:], start=(k == 0), stop=(k == KF - 1))
                for k in range(KF):
                    nc.tensor.matmul(pb[:mw, :], lhsT=bigT[KF + k][:, ms:ms + mw],
                                     rhs=w1b_t[k][:], start=(k == 0), stop=(k == KF - 1))
                uab = sb.tile([P, H2], TBL_DT, tag="uab")
                nc.scalar.activation(out=uab[:mw, :H], in_=pa[:mw, :],
                                     func=mybir.ActivationFunctionType.Copy)
                nc.scalar.activation(out=uab[:mw, H:], in_=pb[:mw, :],
                                     func=mybir.ActivationFunctionType.Copy)
                nc.sync.dma_start(out=u_loc[ms:ms + mw, :], in_=uab[:mw, :])

            # ---------------- Phase B: AllGather u ------------------------
            if not single_core:
                nc.gpsimd.collective_compute(
                    "AllGather", mybir.AluOpType.bypass, replica_groups=groups,
                    ins=[u_loc[:]], outs=[U[:]])

            # ---------------- spmm emitter --------------------------------
            def emit_spmm(table, bias_t, relu, outT, mtag, tdt):
                """outT: list of 2*KH big tiles receiving feature-major result."""
                for t in range(n_tiles):
                    ts_, tw = mtile(t)
                    wl = wins_of_tile[t]
                    ph = ps_big.tile([P, H2], dt.float32, tag="ps_big")
                    nlo = int(clo_base[wl[-1] + 1] - clo_base[wl[0]])
                    nhi = int(chi_base[wl[-1] + 1] - chi_base[wl[0]])
                    nch = nlo + nhi
                    cb = int(clo_base[wl[0]] + chi_base[wl[0]])
                    mt = mpool.tile([P, nch * 32], dt.bfloat16, tag="mt")
                    nc.sync.dma_start(out=mt[:], in_=Mt[:, cb * 32:(cb + nch) * 32])
                    msg = msgp.tile([P, nch * H2], tdt, tag="msg")
                    GMAX = 6   # chunks per dma_gather
                    # lo gathers
                    for a in range(0, nlo, GMAX):
                        b = min(a + GMAX, nlo)
                        o = (int(clo_base[wl[0]]) + a) * 8
                        nc.gpsimd.dma_gather(
                            out_ap=msg[:, a * H2:b * H2].rearrange(
                                "p (n e) -> p n e", e=H2),
                            in_ap=table[:HALF, :],
                            idxs_ap=idxl_t[:, o:o + (b - a) * 8],
                            num_idxs=(b - a) * P, num_idxs_reg=(b - a) * P,
                            elem_size=H2)
                    # hi gathers
                    for a in range(0, nhi, GMAX):
                        b = min(a + GMAX, nhi)
                        o = (int(chi_base[wl[0]]) + a) * 8
                        nc.gpsimd.dma_gather(
                            out_ap=msg[:, (nlo + a) * H2:(nlo + b) * H2].rearrange(
                                "p (n e) -> p n e", e=H2),
                            in_ap=table[HALF:, :],
                            idxs_ap=idxh_t[:, o:o + (b - a) * 8],
                            num_idxs=(b - a) * P, num_idxs_reg=(b - a) * P,
                            elem_size=H2)
                    # per-window chunk lists (positions within the M/msg block)
                    seqs = []
                    for qi, wg in enumerate(wl):
                        lo0 = int(clo_base[wg] - clo_base[wl[0]])
                        hi0 = nlo + int(chi_base[wg] - chi_base[wl[0]])
                        seqs.append([lo0 + k for k in range(int(cpw_lo[wg]))]
                                    + [hi0 + k for k in range(int(cpw_hi[wg]))])
                    # interleave windows round-robin so consecutive PE matmuls
                    # hit different 32-col array strips (concurrent execution)
                    for k in range(max(len(s) for s in seqs)):
                        for qi, s in enumerate(seqs):
                            if k < len(s):
                                j = s[k]
                                nc.tensor.matmul(
                                    ph[32 * qi:32 * qi + 32, :],
                                    lhsT=mt[:, j * 32:(j + 1) * 32],
                                    rhs=msg[:, j * H2:(j + 1) * H2],
                                    start=(k == 0), stop=(k == len(s) - 1),
                                    tile_position=(0, 32 * qi),
                                    skip_group_check=True)
                    hab = sb.tile([P, H2], dt.bfloat16, tag=mtag)
                    nc.vector.tensor_tensor(out=hab[:tw, :], in0=ph[:tw, :],
                                            in1=bias_t[:tw, :],
                                            op=mybir.AluOpType.add)
                    if relu:
                        nc.vector.tensor_scalar_max(hab[:tw, :], hab[:tw, :], 0.0)
                    for fc in range(2 * KH):
                        pt = ps_t.tile([P, P], dt.bfloat16, tag="ps_t")
                        nc.tensor.transpose(out=pt[:, :tw],
                                            in_=hab[:tw, fc * P:(fc + 1) * P],
                                            identity=ident[:tw, :tw])
                        nc.scalar.activation(
                            out=outT[fc][:, ts_:ts_ + tw], in_=pt[:, :tw],
                            func=mybir.ActivationFunctionType.Copy)

            for _rep in range(rep):
                # ---------------- Phase C: h = relu(spmm(U) + b1) -------------
                haT = bigT[0:KH]
                hbT = bigT[KH:2 * KH]
                emit_spmm(U, b1_t, True, haT + hbT, "hab", TBL_DT)

                # ---------------- Phase D: v = h @ W2 -------------------------
                for m in range(n_tiles):
                    ms, mw = mtile(m)
                    pa = ps_d.tile([P, H], dt.float32, tag="ps_d")
                    pb = ps_d.tile([P, H], dt.float32, tag="ps_d")
                    for k in range(KH):
                        nc.tensor.matmul(pa[:mw, :], lhsT=haT[k][:, ms:ms + mw],
                                         rhs=w2a_t[k][:], start=(k == 0), stop=(k == KH - 1))
                    for k in range(KH):
                        nc.tensor.matmul(pb[:mw, :], lhsT=hbT[k][:, ms:ms + mw],
                                         rhs=w2b_t[k][:], start=(k == 0), stop=(k == KH - 1))
                    vab = sb.tile([P, H2], TBL2_DT, tag="vab")
                    nc.scalar.activation(out=vab[:mw, :H], in_=pa[:mw, :],
                                         func=mybir.ActivationFunctionType.Copy)
                    nc.scalar.activation(out=vab[:mw, H:], in_=pb[:mw, :],
                                         func=mybir.ActivationFunctionType.Copy)
                    nc.sync.dma_start(out=v_loc[ms:ms + mw, :], in_=vab[:mw, :])

                # ---------------- Phase E: AllGather v ------------------------
                if not single_core:
                    nc.gpsimd.collective_compute(
                        "AllGather", mybir.AluOpType.bypass, replica_groups=groups,
                        ins=[v_loc[:]], outs=[V[:]])

                # ---------------- Phase F: g = spmm(V) + b2 -------------------
                gaT = bigT[2 * KH:3 * KH]
                gbT = bigT[3 * KH:4 * KH]
                emit_spmm(V, b2_t, False, gaT + gbT, "gab", TBL2_DT)

                # ---------------- Phase G: z = log_softmax(g @ LW + Lb) -------
                zT = bigT[0:2 * KH]

                def softmax_z(py, lb_t, zdst, mw, width):
                    """zdst <- log_softmax(py + lb) ; py is PSUM [P, width] f32."""
                    yf = sb.tile([P, width], dt.float32, tag=f"yf{width}")
                    nc.vector.tensor_tensor(out=yf[:mw, :], in0=py[:mw, :],
                                            in1=lb_t[:mw, :], op=mybir.AluOpType.add)
                    nmx = stat.tile([P, 1], dt.float32, tag="nmx")
                    nc.vector.tensor_reduce(out=nmx[:mw, :], in_=yf[:mw, :],
                                            axis=mybir.AxisListType.X,
                                            op=mybir.AluOpType.max, negate=True)
                    ex = sb.tile([P, width], dt.float32, tag=f"ex{width}")
                    sx = stat.tile([P, 1], dt.float32, tag="sx")
                    nc.scalar.activation(out=ex[:mw, :], in_=yf[:mw, :],
                                         func=mybir.ActivationFunctionType.Exp,
                                         bias=nmx[:mw, :], scale=1.0,
                                         accum_out=sx[:mw, :])
                    lse = stat.tile([P, 1], dt.float32, tag="lse")
                    nc.scalar.activation(out=lse[:mw, :], in_=sx[:mw, :],
                                         func=mybir.ActivationFunctionType.Ln)
                    nc.vector.tensor_scalar(out=zdst, in0=yf[:mw, :],
                                            scalar1=nmx[:mw, :], scalar2=lse[:mw, :],
                                            op0=mybir.AluOpType.add,
                                            op1=mybir.AluOpType.subtract)

                for m in range(n_tiles):
                    ms, mw = mtile(m)
                    zab = sb.tile([P, H2], dt.bfloat16, tag="zab")
                    for br, (gT, lw_t, lb_t) in enumerate(
                            ((gaT, lwa_t, lba_t), (gbT, lwb_t, lbb_t))):
                        py = ps_d.tile([P, H], dt.float32, tag="ps_d")
                        for k in range(KH):
                            nc.tensor.matmul(py[:mw, :], lhsT=gT[k][:, ms:ms + mw],
                                             rhs=lw_t[k][:], start=(k == 0),
                                             stop=(k == KH - 1))
                        softmax_z(py, lb_t, zab[:mw, br * H:(br + 1) * H], mw, H)
                    for fc in range(2 * KH):
                        pt = ps_t.tile([P, P], dt.bfloat16, tag="ps_t")
                        nc.tensor.transpose(out=pt[:, :mw],
                                            in_=zab[:mw, fc * P:(fc + 1) * P],
                                            identity=ident[:mw, :mw])
                        nc.scalar.activation(
                            out=zT[fc][:, ms:ms + mw], in_=pt[:, :mw],
                            func=mybir.ActivationFunctionType.Copy)

                # ---------------- Phase H: out = log_softmax(z @ LWf + Lb) ----
                for m in range(n_tiles):
                    ms, mw = mtile(m)
                    pf = ps_f.tile([P, C], dt.float32, tag="ps_f")
                    for k in range(2 * KH):
                        nc.tensor.matmul(pf[:mw, :], lhsT=zT[k][:, ms:ms + mw],
                                         rhs=lwf_t[k][:], start=(k == 0),
                                         stop=(k == 2 * KH - 1))
                    ot = sb.tile([P, C], dt.float32, tag="ot")
                    softmax_z(pf, lbf_t, ot[:mw, :], mw, C)
                    nc.sync.dma_start(out=out_t[ms:ms + mw, :], in_=ot[:mw, :])

    import os
    if os.environ.get("NO_ACT_PIN"):
        nc.compile()
    else:
        with _pinned_act_tables():
            nc.compile()
    return nc


# ----------------------------------------------------------------------------
# Entry point
# ----------------------------------------------------------------------------

_CACHE = {}


def kernel(x0, x1, edge_src, edge_dst, edge_w,
           W1a, b1a, W2a, b2a, LWa, Lba,
           W1b, b1b, W2b, b2b, LWb, Lbb,
           LW, Lb):
    x0 = np.asarray(x0)
    x1 = np.asarray(x1)
    N, F0 = x0.shape
    H = np.asarray(W1a).shape[1]
    C = np.asarray(LW).shape[1]
    S = N // N_CORES

    key = (N, F0, H, C,
           hash(np.asarray(edge_src).tobytes()) ^ hash(np.asarray(edge_dst).tobytes()))
    if key not in _CACHE:
        cpw_lo, cpw_hi, M_list, idxl_list, idxh_list = preprocess_edges(
            edge_src, edge_dst, edge_w, N, S)
        nc = build_nc(N, F0, H, C, S, cpw_lo, cpw_hi)
        _CACHE[key] = (nc, M_list, idxl_list, idxh_list)
    nc, M_list, idxl_list, idxh_list = _CACHE[key]

    bf = lambda a: np.asarray(a, dtype=BF16)
    f32 = lambda a: np.asarray(a, dtype=np.float32)
    bcast = lambda v: np.broadcast_to(np.asarray(v, dtype=BF16)[None, :], (P, len(v))).copy()

    x0T = bf(x0).T
    x1T = bf(x1).T
    shared = {
        "W1a": bf(W1a), "W1b": bf(W1b), "W2a": bf(W2a), "W2b": bf(W2b),
        "LWa": bf(LWa), "LWb": bf(LWb), "LWf": bf(LW),
        "b1": bcast(np.concatenate([f32(b1a), f32(b1b)])),
        "b2": bcast(np.concatenate([f32(b2a), f32(b2b)])),
        "lba": bcast(f32(Lba)), "lbb": bcast(f32(Lbb)), "lbf": bcast(f32(Lb)),
    }
    in_maps = []
    for c in range(N_CORES):
        in_maps.append({
            **shared,
            "x0T": np.ascontiguousarray(x0T[:, c * S:(c + 1) * S]),
            "x1T": np.ascontiguousarray(x1T[:, c * S:(c + 1) * S]),
            "M": M_list[c], "IDXL": idxl_list[c], "IDXH": idxh_list[c],
        })
    res = run_bass_kernel_spmd(nc, in_maps, list(range(N_CORES)))
    return np.concatenate([res.results[c]["out"] for c in range(N_CORES)], axis=0)



# revision 5
# speedup vs baseline: 1.8445x; 1.8445x over previous
"""Bass/Trainium2 kernel for the 2-branch GCN (gnn_message_passing).

Computation (reference):
    per branch i in {a, b}:
        u_i = x_i @ W1_i                                  [N, H]
        h_i = relu(spmm(A, u_i) + b1_i)                   [N, H]
        v_i = h_i @ W2_i                                  [N, H]
        g_i = spmm(A, v_i) + b2_i                         [N, H]
        z_i = log_softmax(g_i @ LW_i + Lb_i)              [N, H]
    out = log_softmax(concat(z_a, z_b) @ LW + Lb)         [N, C]
where spmm(A, u)[d] = sum_{e: dst[e]=d} w[e] * u[src[e]].

Strategy (8 NeuronCores, node-sharded, fp8 message path):
  - Core c owns node rows [c*S, (c+1)*S), S = N/8.  Dense matmuls in bf16.
  - Activation tables U = allgather(x@W1), V = allgather(h@W2) stored fp8e4
    (concat a|b features -> 512B rows); both spmm layers gather rows of the
    concat table once per edge (512B descriptors, the 1x-latency minimum).
  - Edges grouped per (dst 128-tile, src half); chunk counts are padded to
    the max across the 8 cores so the compiled program is shared (SPMD).
  - Aggregation: one-hot matrices M (fp8, edge weight at the dst column)
    multiply gathered messages on the PE.  Chunk pairs whose 256 edges fall
    in one 64-dst window on ALL cores use a single DoubleRow fp8 matmul
    (0.5 cycles/row); mixed pairs emit two window-masked DoubleRow matmuls;
    a trailing odd chunk uses a plain [128,128] fp8 matmul.
  - Bias rides a K=1 matmul (ones x bias row) that also opens (start=True)
    each 64-row PSUM region; relu/cast psum->SBUF is one ACT op.
  - Feature-major activations live in two [128, 4S] SBUF tiles (h, g, z
    reuse the x0/x1 space); writeback per tile = 4 PE transposes into one
    PSUM bank + one 4-block strided ACT copy.
"""

import sys

if "/opt/trn_rl_repo" not in sys.path:
    sys.path.insert(0, "/opt/trn_rl_repo")

import numpy as np
import ml_dtypes

import concourse.bass as bass
import concourse.bacc as bacc
import concourse.mybir as mybir
import concourse.tile as tile
from concourse.tile import TileContext
from concourse.masks import make_identity
from concourse.bass_utils import run_bass_kernel_spmd

import contextlib
import concourse.bacc as _bacc_mod


@contextlib.contextmanager
def _pinned_act_tables():
    """During compile, make every activation-function table except the
    all-purpose one look empty so bacc's table-load inserter picks a single
    table for the whole program (one LoadActFuncSet instead of ~300)."""
    orig = _bacc_mod.get_activation_tables

    def pinned(arch):
        tabs = orig(arch)
        keep = "natural_log_exp_and_others"
        if keep in tabs:
            tabs = {k: (v if k == keep else set()) for k, v in tabs.items()}
        return tabs

    _bacc_mod.get_activation_tables = pinned
    try:
        yield
    finally:
        _bacc_mod.get_activation_tables = orig


BF16 = ml_dtypes.bfloat16
F8 = ml_dtypes.float8_e4m3
dt = mybir.dt
P = 128
N_CORES = 8
TBL_DT = dt.float8e4          # gather-table / message / M dtype


# ----------------------------------------------------------------------------
# Host-side edge preprocessing
# ----------------------------------------------------------------------------

def preprocess_edges(edge_src, edge_dst, edge_w, N, S):
    """Group edges per (dst 128-tile, src half), sorted by dst within each
    group.  Chunk = 128 gather slots; slot k*128+p holds sorted edge k*128+p.

    Emission plan (shared across cores):
      per (tile, half): for each pair of chunks j -> one DoubleRow matmul if
      the pair's edges lie in one 64-dst window on every core ("pure"), else
      two window-masked DoubleRow matmuls; a trailing odd chunk -> one plain
      [128,128] matmul.

    Returns (plan, M_list, idxl_list, idxh_list).
    """
    edge_src = np.asarray(edge_src).astype(np.int64)
    edge_dst = np.asarray(edge_dst).astype(np.int64)
    edge_w = np.asarray(edge_w, dtype=np.float32)
    n_tiles = (S + P - 1) // P
    HALF = N // 2

    per_core = []
    cnt = np.zeros((N_CORES, n_tiles, 2), dtype=np.int64)
    for c in range(N_CORES):
        sel = (edge_dst >= c * S) & (edge_dst < (c + 1) * S)
        dl = edge_dst[sel] - c * S
        sg = edge_src[sel]
        w = edge_w[sel]
        hi = (sg >= HALF).astype(np.int64)
        t = dl >> 7
        order = np.lexsort((dl, hi, t))
        dl, sg, w, hi, t = dl[order], sg[order], w[order], hi[order], t[order]
        gid = t * 2 + hi
        g = np.bincount(gid, minlength=2 * n_tiles)
        cnt[c] = g.reshape(n_tiles, 2)
        gstart = np.concatenate([[0], np.cumsum(g)])
        per_core.append((dl, sg, w, gid, gstart))

    cpw = np.maximum(1, (cnt.max(axis=0) + P - 1) // P)   # [n_tiles, 2]

    # ---- emission plan ----------------------------------------------------
    # blocks[t] = list of (h, kind, idx, w) in emission order; kind in
    # {"dr", "fat"}; idx = pair index j (dr) or chunk index k (fat);
    # w = 64-dst window (dr only; None for mixed covered via two entries).
    blocks = []
    nblk = np.zeros(n_tiles, dtype=np.int64)
    for t in range(n_tiles):
        bl = []
        for h in (0, 1):
            npair = int(cpw[t, h]) // 2
            odd = int(cpw[t, h]) % 2
            for j in range(npair):
                # pure if, on every core, all real edges of pair j fall in
                # one 64-window
                wset = set()
                for c in range(N_CORES):
                    dl, sg, w_, gid, gstart = per_core[c]
                    g0 = gstart[2 * t + h]
                    n = cnt[c, t, h]
                    a = min(256 * j, n)
                    b = min(256 * (j + 1), n)
                    if b > a:
                        dloc = dl[g0 + a:g0 + b] - 128 * t
                        if (dloc < 64).any():
                            wset.add(0)
                        if (dloc >= 64).any():
                            wset.add(1)
                if len(wset) <= 1:
                    bl.append((h, "dr", j, wset.pop() if wset else 0))
                else:
                    bl.append((h, "dr", j, 0))
                    bl.append((h, "dr", j, 1))
            if odd:
                bl.append((h, "fat", int(cpw[t, h]) - 1, None))
        blocks.append(bl)
        nblk[t] = len(bl)
    mblk_base = np.concatenate([[0], np.cumsum(nblk)])
    NBLK = int(mblk_base[-1])

    clo_base = np.concatenate([[0], np.cumsum(cpw[:, 0])])
    chi_base = np.concatenate([[0], np.cumsum(cpw[:, 1])])
    CLO, CHI = int(clo_base[-1]), int(chi_base[-1])

    # ---- per-core M / idx tensors ----------------------------------------
    M_list, idxl_list, idxh_list = [], [], []
    for c in range(N_CORES):
        dl, sg, w_, gid, gstart = per_core[c]
        M = np.zeros((P, NBLK * P), dtype=F8)
        idxl = np.zeros((P, CLO * 8), dtype=np.int16)
        idxh = np.zeros((P, CHI * 8), dtype=np.int16)
        for t in range(n_tiles):
            for bi, (h, kind, idx, wwin) in enumerate(blocks[t]):
                g0 = gstart[2 * t + h]
                n = int(cnt[c, t, h])
                col0 = (int(mblk_base[t]) + bi) * P
                if kind == "dr":
                    a = min(256 * idx, n)
                    b = min(256 * (idx + 1), n)
                    if b <= a:
                        continue
                    r = np.arange(a, b)
                    dloc = dl[g0 + a:g0 + b] - 128 * t
                    sel = (dloc >= 64) == (wwin == 1)
                    r, dloc = r[sel], dloc[sel]
                    i = (r - 256 * idx) >> 7
                    p = r & 127
                    M[p, col0 + i * 64 + (dloc - 64 * wwin)] = \
                        w_[g0 + r].astype(F8)
                else:  # fat
                    a = min(128 * idx, n)
                    b = min(128 * (idx + 1), n)
                    if b <= a:
                        continue
                    r = np.arange(a, b)
                    dloc = dl[g0 + a:g0 + b] - 128 * t
                    M[r & 127, col0 + dloc] = w_[g0 + r].astype(F8)
            # idx arrays: chunk k slot p -> sorted edge k*128+p (pad -> 0)
            for h, arr, base, off in ((0, idxl, clo_base, 0),
                                      (1, idxh, chi_base, HALF)):
                g0 = gstart[2 * t + h]
                n = int(cnt[c, t, h])
                nck = int(cpw[t, h])
                vals = np.zeros(nck * P, dtype=np.int16)
                vals[:n] = (sg[g0:g0 + n] - off).astype(np.int16)
                # wrapped layout: slot s -> row s%16 (replicated x8), col s//16
                cols = int(base[t]) * 8 + (np.arange(nck * P) >> 4)
                rows = np.arange(nck * P) & 15
                for g in range(8):
                    arr[16 * g + rows, cols] = vals
        M_list.append(M)
        idxl_list.append(idxl)
        idxh_list.append(idxh)

    plan = {
        "cpw": cpw, "blocks": blocks, "nblk": nblk, "mblk_base": mblk_base,
        "NBLK": NBLK, "clo_base": clo_base, "chi_base": chi_base,
        "CLO": CLO, "CHI": CHI, "n_tiles": n_tiles,
    }
    return plan, M_list, idxl_list, idxh_list


# ----------------------------------------------------------------------------
# Bass program
# ----------------------------------------------------------------------------

def build_nc(N, F0, H, C, S, plan, single_core=False):
    n_tiles = plan["n_tiles"]
    cpw = plan["cpw"]
    blocks = plan["blocks"]
    mblk_base = plan["mblk_base"]
    NBLK = plan["NBLK"]
    clo_base = plan["clo_base"]
    chi_base = plan["chi_base"]
    CLO, CHI = plan["CLO"], plan["CHI"]
    HALF = N // 2
    KF = F0 // P       # k-chunks of F0 (4)
    KH = H // P        # k-chunks of H (2)
    H2 = 2 * H
    DR = mybir.MatmulPerfMode.DoubleRow

    nc = bacc.Bacc("TRN2", num_devices=1 if single_core else N_CORES,
                   dynamic_dma_scratch_size=36864)

    # --- I/O ---
    x0T = nc.declare_dram_parameter("x0T", [F0, S], dt.bfloat16, isOutput=False)
    x1T = nc.declare_dram_parameter("x1T", [F0, S], dt.bfloat16, isOutput=False)
    W1a = nc.declare_dram_parameter("W1a", [F0, H], dt.bfloat16, isOutput=False)
    W1b = nc.declare_dram_parameter("W1b", [F0, H], dt.bfloat16, isOutput=False)
    W2a = nc.declare_dram_parameter("W2a", [H, H], dt.bfloat16, isOutput=False)
    W2b = nc.declare_dram_parameter("W2b", [H, H], dt.bfloat16, isOutput=False)
    LWa = nc.declare_dram_parameter("LWa", [H, H], dt.bfloat16, isOutput=False)
    LWb = nc.declare_dram_parameter("LWb", [H, H], dt.bfloat16, isOutput=False)
    LWf = nc.declare_dram_parameter("LWf", [H2, C], dt.bfloat16, isOutput=False)
    b1 = nc.declare_dram_parameter("b1", [P, H2], dt.bfloat16, isOutput=False)
    b2 = nc.declare_dram_parameter("b2", [P, H2], dt.bfloat16, isOutput=False)
    lba = nc.declare_dram_parameter("lba", [P, H], dt.bfloat16, isOutput=False)
    lbb = nc.declare_dram_parameter("lbb", [P, H], dt.bfloat16, isOutput=False)
    lbf = nc.declare_dram_parameter("lbf", [P, C], dt.bfloat16, isOutput=False)
    Mt = nc.declare_dram_parameter("M", [P, NBLK * P], TBL_DT, isOutput=False)
    IDXL = nc.declare_dram_parameter("IDXL", [P, CLO * 8], dt.int16, isOutput=False)
    IDXH = nc.declare_dram_parameter("IDXH", [P, CHI * 8], dt.int16, isOutput=False)
    out_t = nc.declare_dram_parameter("out", [S, C], dt.float32, isOutput=True)

    # --- internal DRAM ---
    u_loc = nc.dram_tensor("u_loc", [S, H2], TBL_DT)
    v_loc = nc.dram_tensor("v_loc", [S, H2], TBL_DT)
    if single_core:
        U = nc.declare_dram_parameter("Uin", [N, H2], TBL_DT, isOutput=False)
        V = nc.declare_dram_parameter("Vin", [N, H2], TBL_DT, isOutput=False)
    else:
        U = nc.dram_tensor("U", [N, H2], TBL_DT, addr_space="Shared")
        V = nc.dram_tensor("V", [N, H2], TBL_DT, addr_space="Shared")
    groups = [list(range(N_CORES))]

    with TileContext(nc, num_cores=N_CORES) as tc:
        ctx = contextlib.ExitStack()
        with ctx:
            perm = ctx.enter_context(tc.tile_pool(name="perm", bufs=1))
            big = ctx.enter_context(tc.tile_pool(name="big", bufs=1))
            mpool = ctx.enter_context(tc.tile_pool(name="mpool", bufs=2))
            msgp = ctx.enter_context(tc.tile_pool(name="msgp", bufs=2))
            sb = ctx.enter_context(tc.tile_pool(name="sb", bufs=2))
            stat = ctx.enter_context(tc.tile_pool(name="stat", bufs=4))
            ps_big = ctx.enter_context(tc.tile_pool(name="ps_big", bufs=2, space="PSUM"))
            ps_d = ctx.enter_context(tc.tile_pool(name="ps_d", bufs=2, space="PSUM"))
            ps_t = ctx.enter_context(tc.tile_pool(name="ps_t", bufs=2, space="PSUM"))
            ps_f = ctx.enter_context(tc.tile_pool(name="ps_f", bufs=2, space="PSUM"))

            # persistent small tiles
            ident = perm.tile([P, P], dt.bfloat16, tag="ident")
            make_identity(nc, ident[:])
            ones_t = perm.tile([P, P], dt.bfloat16, tag="ones")
            nc.vector.memset(ones_t[:], 1.0)
            w1a_t = [perm.tile([P, H], dt.bfloat16, name=f"w1a{k}", tag=f"w1a{k}") for k in range(KF)]
            w1b_t = [perm.tile([P, H], dt.bfloat16, name=f"w1b{k}", tag=f"w1b{k}") for k in range(KF)]
            w2a_t = [perm.tile([P, H], dt.bfloat16, name=f"w2a{k}", tag=f"w2a{k}") for k in range(KH)]
            w2b_t = [perm.tile([P, H], dt.bfloat16, name=f"w2b{k}", tag=f"w2b{k}") for k in range(KH)]
            lwa_t = [perm.tile([P, H], dt.bfloat16, name=f"lwa{k}", tag=f"lwa{k}") for k in range(KH)]
            lwb_t = [perm.tile([P, H], dt.bfloat16, name=f"lwb{k}", tag=f"lwb{k}") for k in range(KH)]
            lwf_t = [perm.tile([P, C], dt.bfloat16, name=f"lwf{k}", tag=f"lwf{k}") for k in range(2 * KH)]
            for k in range(KF):
                nc.sync.dma_start(out=w1a_t[k][:], in_=W1a[k * P:(k + 1) * P, :])
                nc.sync.dma_start(out=w1b_t[k][:], in_=W1b[k * P:(k + 1) * P, :])
            for k in range(KH):
                nc.sync.dma_start(out=w2a_t[k][:], in_=W2a[k * P:(k + 1) * P, :])
                nc.sync.dma_start(out=w2b_t[k][:], in_=W2b[k * P:(k + 1) * P, :])
                nc.sync.dma_start(out=lwa_t[k][:], in_=LWa[k * P:(k + 1) * P, :])
                nc.sync.dma_start(out=lwb_t[k][:], in_=LWb[k * P:(k + 1) * P, :])
            for k in range(2 * KH):
                nc.sync.dma_start(out=lwf_t[k][:], in_=LWf[k * P:(k + 1) * P, :])
            b1_t = perm.tile([P, H2], dt.bfloat16, tag="b1")
            b2_t = perm.tile([P, H2], dt.bfloat16, tag="b2")
            lba_t = perm.tile([P, H], dt.bfloat16, tag="lba")
            lbb_t = perm.tile([P, H], dt.bfloat16, tag="lbb")
            lbf_t = perm.tile([P, C], dt.bfloat16, tag="lbf")
            nc.sync.dma_start(out=b1_t[:], in_=b1[:])
            nc.sync.dma_start(out=b2_t[:], in_=b2[:])
            nc.sync.dma_start(out=lba_t[:], in_=lba[:])
            nc.sync.dma_start(out=lbb_t[:], in_=lbb[:])
            nc.sync.dma_start(out=lbf_t[:], in_=lbf[:])
            idxl_t = perm.tile([P, CLO * 8], dt.int16, tag="idxl")
            nc.sync.dma_start(out=idxl_t[:], in_=IDXL[:])
            idxh_t = perm.tile([P, CHI * 8], dt.int16, tag="idxh")
            nc.sync.dma_start(out=idxh_t[:], in_=IDXH[:])

            # two big feature-major tiles [P, 4S]; reused across phases:
            #   phase A in: big0 = x0T (4 k-chunks), big1 = x1T
            #   phase C out: big0 = hT (ha0 ha1 hb0 hb1 chunk-major)
            #   phase F out: big1 = gT
            #   phase G out: big0 = zT
            big0 = big.tile([P, 4 * S], dt.bfloat16, tag="big0")
            big1 = big.tile([P, 4 * S], dt.bfloat16, tag="big1")
            for k in range(KF):
                nc.sync.dma_start(out=big0[:, k * S:(k + 1) * S],
                                  in_=x0T[k * P:(k + 1) * P, :])
                nc.sync.dma_start(out=big1[:, k * S:(k + 1) * S],
                                  in_=x1T[k * P:(k + 1) * P, :])

            def mtile(m):
                ms = m * P
                return ms, min(P, S - ms)

            # ---------------- Phase A: u = x @ W1 (both branches) ----------
            for m in range(n_tiles):
                ms, mw = mtile(m)
                pa = ps_d.tile([P, H], dt.float32, tag="ps_d")
                pb = ps_d.tile([P, H], dt.float32, tag="ps_d")
                for k in range(KF):
                    nc.tensor.matmul(pa[:mw, :], lhsT=big0[:, k * S + ms:k * S + ms + mw],
                                     rhs=w1a_t[k][:], start=(k == 0), stop=(k == KF - 1))
                for k in range(KF):
                    nc.tensor.matmul(pb[:mw, :], lhsT=big1[:, k * S + ms:k * S + ms + mw],
                                     rhs=w1b_t[k][:], start=(k == 0), stop=(k == KF - 1))
                uab = sb.tile([P, H2], TBL_DT, tag="uab")
                nc.scalar.activation(out=uab[:mw, :H], in_=pa[:mw, :],
                                     func=mybir.ActivationFunctionType.Copy)
                nc.scalar.activation(out=uab[:mw, H:], in_=pb[:mw, :],
                                     func=mybir.ActivationFunctionType.Copy)
                nc.sync.dma_start(out=u_loc[ms:ms + mw, :], in_=uab[:mw, :])

            # ---------------- Phase B: AllGather u ------------------------
            if not single_core:
                nc.gpsimd.collective_compute(
                    "AllGather", mybir.AluOpType.bypass, replica_groups=groups,
                    ins=[u_loc[:]], outs=[U[:]])

            # ---------------- spmm emitter --------------------------------
            def emit_spmm(table, bias_t, relu, outT):
                """outT: [P, 4S] tile receiving feature-major result."""
                for t in range(n_tiles):
                    ts_, tw = mtile(t)
                    nlo, nhi = int(cpw[t, 0]), int(cpw[t, 1])
                    nch = nlo + nhi
                    bl = blocks[t]
                    nb = len(bl)
                    mb0 = int(mblk_base[t])
                    ph = ps_big.tile([P, H2], dt.float32, tag="ps_big")
                    mt = mpool.tile([P, nb * P], TBL_DT, tag="mt")
                    nc.sync.dma_start(out=mt[:], in_=Mt[:, mb0 * P:(mb0 + nb) * P])
                    msg = msgp.tile([P, nch * H2], TBL_DT, tag="msg")
                    for h, n_k, base, it in ((0, nlo, clo_base, idxl_t),
                                             (1, nhi, chi_base, idxh_t)):
                        o = int(base[t]) * 8
                        co = 0 if h == 0 else nlo
                        nc.gpsimd.dma_gather(
                            out_ap=msg[:, co * H2:(co + n_k) * H2].rearrange(
                                "p (n e) -> p n e", e=H2),
                            in_ap=table[:HALF, :] if h == 0 else table[HALF:, :],
                            idxs_ap=it[:, o:o + n_k * 8],
                            num_idxs=n_k * P, num_idxs_reg=n_k * P,
                            elem_size=H2)
                    # bias openers (start=True zeroes each 64-row region)
                    nc.tensor.matmul(ph[0:64, :], lhsT=ones_t[0:1, 0:64],
                                     rhs=bias_t[0:1, :], start=True, stop=False,
                                     skip_group_check=True)
                    nc.tensor.matmul(ph[64:128, :], lhsT=ones_t[0:1, 0:64],
                                     rhs=bias_t[0:1, :], start=True, stop=False,
                                     skip_group_check=True)
                    for bi, (h, kind, idx, wwin) in enumerate(bl):
                        co = 0 if h == 0 else nlo
                        last = (bi == nb - 1)
                        if kind == "dr":
                            nc.tensor.matmul(
                                ph[64 * wwin:64 * wwin + 64, :],
                                lhsT=mt[:, bi * P:(bi + 1) * P].rearrange(
                                    "p (i d) -> p i d", i=2),
                                rhs=msg[:, (co + 2 * idx) * H2:(co + 2 * idx + 2) * H2
                                        ].rearrange("p (i e) -> p i e", i=2),
                                start=False, stop=last, perf_mode=DR,
                                skip_group_check=True)
                        else:
                            nc.tensor.matmul(
                                ph[:, :],
                                lhsT=mt[:, bi * P:(bi + 1) * P],
                                rhs=msg[:, (co + idx) * H2:(co + idx + 1) * H2],
                                start=False, stop=last,
                                skip_group_check=True)
                    hab = sb.tile([P, H2], dt.bfloat16, tag="hab")
                    nc.scalar.activation(
                        out=hab[:tw, :], in_=ph[:tw, :],
                        func=(mybir.ActivationFunctionType.Relu if relu
                              else mybir.ActivationFunctionType.Copy))
                    pt = ps_t.tile([P, H2], dt.bfloat16, tag="ps_t")
                    for fc in range(2 * KH):
                        nc.tensor.transpose(out=pt[:, fc * P:fc * P + tw],
                                            in_=hab[:tw, fc * P:(fc + 1) * P],
                                            identity=ident[:tw, :tw])
                    nc.scalar.activation(
                        out=outT[:, :].rearrange("p (f s) -> p f s", f=4)[:, :, ts_:ts_ + tw],
                        in_=pt[:, :].rearrange("p (f s) -> p f s", f=4)[:, :, :tw],
                        func=mybir.ActivationFunctionType.Copy)

            # ---------------- Phase C: h = relu(spmm(U) + b1) -------------
            emit_spmm(U, b1_t, True, big0)

            # ---------------- Phase D: v = h @ W2 -------------------------
            for m in range(n_tiles):
                ms, mw = mtile(m)
                pa = ps_d.tile([P, H], dt.float32, tag="ps_d")
                pb = ps_d.tile([P, H], dt.float32, tag="ps_d")
                for k in range(KH):
                    nc.tensor.matmul(pa[:mw, :], lhsT=big0[:, k * S + ms:k * S + ms + mw],
                                     rhs=w2a_t[k][:], start=(k == 0), stop=(k == KH - 1))
                for k in range(KH):
                    nc.tensor.matmul(pb[:mw, :],
                                     lhsT=big0[:, (KH + k) * S + ms:(KH + k) * S + ms + mw],
                                     rhs=w2b_t[k][:], start=(k == 0), stop=(k == KH - 1))
                vab = sb.tile([P, H2], TBL_DT, tag="vab")
                nc.scalar.activation(out=vab[:mw, :H], in_=pa[:mw, :],
                                     func=mybir.ActivationFunctionType.Copy)
                nc.scalar.activation(out=vab[:mw, H:], in_=pb[:mw, :],
                                     func=mybir.ActivationFunctionType.Copy)
                nc.sync.dma_start(out=v_loc[ms:ms + mw, :], in_=vab[:mw, :])

            # ---------------- Phase E: AllGather v ------------------------
            if not single_core:
                nc.gpsimd.collective_compute(
                    "AllGather", mybir.AluOpType.bypass, replica_groups=groups,
                    ins=[v_loc[:]], outs=[V[:]])

            # ---------------- Phase F: g = spmm(V) + b2 -------------------
            emit_spmm(V, b2_t, False, big1)

            # ---------------- Phase G: z = log_softmax(g @ LW + Lb) -------
            def softmax_z(py, lb_t, zdst, mw, width):
                """zdst <- log_softmax(py + lb) ; py is PSUM [P, width] f32."""
                yf = sb.tile([P, width], dt.float32, tag=f"yf{width}")
                nc.vector.tensor_tensor(out=yf[:mw, :], in0=py[:mw, :],
                                        in1=lb_t[:mw, :], op=mybir.AluOpType.add)
                nmx = stat.tile([P, 1], dt.float32, tag="nmx")
                nc.vector.tensor_reduce(out=nmx[:mw, :], in_=yf[:mw, :],
                                        axis=mybir.AxisListType.X,
                                        op=mybir.AluOpType.max, negate=True)
                ex = sb.tile([P, width], dt.float32, tag=f"ex{width}")
                sx = stat.tile([P, 1], dt.float32, tag="sx")
                nc.scalar.activation(out=ex[:mw, :], in_=yf[:mw, :],
                                     func=mybir.ActivationFunctionType.Exp,
                                     bias=nmx[:mw, :], scale=1.0,
                                     accum_out=sx[:mw, :])
                lse = stat.tile([P, 1], dt.float32, tag="lse")
                nc.scalar.activation(out=lse[:mw, :], in_=sx[:mw, :],
                                     func=mybir.ActivationFunctionType.Ln)
                nc.vector.tensor_scalar(out=zdst, in0=yf[:mw, :],
                                        scalar1=nmx[:mw, :], scalar2=lse[:mw, :],
                                        op0=mybir.AluOpType.add,
                                        op1=mybir.AluOpType.subtract)

            for m in range(n_tiles):
                ms, mw = mtile(m)
                zab = sb.tile([P, H2], dt.bfloat16, tag="zab")
                for br, (lw_t, lb_t) in enumerate(
                        ((lwa_t, lba_t), (lwb_t, lbb_t))):
                    py = ps_d.tile([P, H], dt.float32, tag="ps_d")
                    for k in range(KH):
                        nc.tensor.matmul(
                            py[:mw, :],
                            lhsT=big1[:, (2 * br + k) * S + ms:(2 * br + k) * S + ms + mw],
                            rhs=lw_t[k][:], start=(k == 0), stop=(k == KH - 1))
                    softmax_z(py, lb_t, zab[:mw, br * H:(br + 1) * H], mw, H)
                pt = ps_t.tile([P, H2], dt.bfloat16, tag="ps_t")
                for fc in range(2 * KH):
                    nc.tensor.transpose(out=pt[:, fc * P:fc * P + mw],
                                        in_=zab[:mw, fc * P:(fc + 1) * P],
                                        identity=ident[:mw, :mw])
                nc.scalar.activation(
                    out=big0[:, :].rearrange("p (f s) -> p f s", f=4)[:, :, ms:ms + mw],
                    in_=pt[:, :].rearrange("p (f s) -> p f s", f=4)[:, :, :mw],
                    func=mybir.ActivationFunctionType.Copy)

            # ---------------- Phase H: out = log_softmax(z @ LWf + Lb) ----
            for m in range(n_tiles):
                ms, mw = mtile(m)
                pf = ps_f.tile([P, C], dt.float32, tag="ps_f")
                for k in range(2 * KH):
                    nc.tensor.matmul(pf[:mw, :],
                                     lhsT=big0[:, k * S + ms:k * S + ms + mw],
                                     rhs=lwf_t[k][:], start=(k == 0),
                                     stop=(k == 2 * KH - 1))
                ot = sb.tile([P, C], dt.float32, tag="ot")
                softmax_z(pf, lbf_t, ot[:mw, :], mw, C)
                nc.sync.dma_start(out=out_t[ms:ms + mw, :], in_=ot[:mw, :])

    import os
    if os.environ.get("NO_ACT_PIN"):
        nc.compile()
    else:
        with _pinned_act_tables():
            nc.compile()
    return nc


# ----------------------------------------------------------------------------
# Entry point
# ----------------------------------------------------------------------------

_CACHE = {}


def kernel(x0, x1, edge_src, edge_dst, edge_w,
           W1a, b1a, W2a, b2a, LWa, Lba,
           W1b, b1b, W2b, b2b, LWb, Lbb,
           LW, Lb):
    x0 = np.asarray(x0)
    x1 = np.asarray(x1)
    N, F0 = x0.shape
    H = np.asarray(W1a).shape[1]
    C = np.asarray(LW).shape[1]
    S = N // N_CORES

    key = (N, F0, H, C,
           hash(np.asarray(edge_src).tobytes()) ^ hash(np.asarray(edge_dst).tobytes()))
    if key not in _CACHE:
        plan, M_list, idxl_list, idxh_list = preprocess_edges(
            edge_src, edge_dst, edge_w, N, S)
        nc = build_nc(N, F0, H, C, S, plan)
        _CACHE[key] = (nc, M_list, idxl_list, idxh_list)
    nc, M_list, idxl_list, idxh_list = _CACHE[key]

    bf = lambda a: np.asarray(a, dtype=BF16)
    f32 = lambda a: np.asarray(a, dtype=np.float32)
    bcast = lambda v: np.broadcast_to(np.asarray(v, dtype=BF16)[None, :], (P, len(v))).copy()

    x0T = bf(x0).T
    x1T = bf(x1).T
    shared = {
        "W1a": bf(W1a), "W1b": bf(W1b), "W2a": bf(W2a), "W2b": bf(W2b),
        "LWa": bf(LWa), "LWb": bf(LWb), "LWf": bf(LW),
        "b1": bcast(np.concatenate([f32(b1a), f32(b1b)])),
        "b2": bcast(np.concatenate([f32(b2a), f32(b2b)])),
        "lba": bcast(f32(Lba)), "lbb": bcast(f32(Lbb)), "lbf": bcast(f32(Lb)),
    }
    in_maps = []
    for c in range(N_CORES):
        in_maps.append({
            **shared,
            "x0T": np.ascontiguousarray(x0T[:, c * S:(c + 1) * S]),
            "x1T": np.ascontiguousarray(x1T[:, c * S:(c + 1) * S]),
            "M": M_list[c], "IDXL": idxl_list[c], "IDXH": idxh_list[c],
        })
    res = run_bass_kernel_spmd(nc, in_maps, list(range(N_CORES)))
    return np.concatenate([res.results[c]["out"] for c in range(N_CORES)], axis=0)


# revision 8
# speedup vs baseline: 1.8776x; 1.0180x over previous
"""Bass/Trainium2 kernel for the 2-branch GCN (gnn_message_passing).

Computation (reference):
    per branch i in {a, b}:
        u_i = x_i @ W1_i                                  [N, H]
        h_i = relu(spmm(A, u_i) + b1_i)                   [N, H]
        v_i = h_i @ W2_i                                  [N, H]
        g_i = spmm(A, v_i) + b2_i                         [N, H]
        z_i = log_softmax(g_i @ LW_i + Lb_i)              [N, H]
    out = log_softmax(concat(z_a, z_b) @ LW + Lb)         [N, C]
where spmm(A, u)[d] = sum_{e: dst[e]=d} w[e] * u[src[e]].

Strategy (8 NeuronCores, node-sharded, fp8 message path):
  - Core c owns node rows [c*S, (c+1)*S), S = N/8.  Dense matmuls in bf16.
  - Activation tables U = allgather(x@W1), V = allgather(h@W2) stored fp8e4
    (concat a|b features -> 512B rows); both spmm layers gather rows of the
    concat table once per edge (512B descriptors, the 1x-latency minimum).
  - Edges grouped per (dst 128-tile, src half); chunk counts are padded to
    the max across the 8 cores so the compiled program is shared (SPMD).
  - Aggregation: one-hot matrices M (fp8, edge weight at the dst column)
    multiply gathered messages on the PE.  Chunk pairs whose 256 edges fall
    in one 64-dst window on ALL cores use a single DoubleRow fp8 matmul
    (0.5 cycles/row); mixed pairs emit two window-masked DoubleRow matmuls;
    a trailing odd chunk uses a plain [128,128] fp8 matmul.
  - Bias rides a K=1 matmul (ones x bias row) that also opens (start=True)
    each 64-row PSUM region; relu/cast psum->SBUF is one ACT op.
  - Feature-major activations live in two [128, 4S] SBUF tiles (h, g, z
    reuse the x0/x1 space); writeback per tile = 4 PE transposes into one
    PSUM bank + one 4-block strided ACT copy.
"""

import sys

if "/opt/trn_rl_repo" not in sys.path:
    sys.path.insert(0, "/opt/trn_rl_repo")

import numpy as np
import ml_dtypes

import concourse.bass as bass
import concourse.bacc as bacc
import concourse.mybir as mybir
import concourse.tile as tile
from concourse.tile import TileContext
from concourse.masks import make_identity
from concourse.bass_utils import run_bass_kernel_spmd

import contextlib
import concourse.bacc as _bacc_mod


@contextlib.contextmanager
def _pinned_act_tables():
    """During compile, make every activation-function table except the
    all-purpose one look empty so bacc's table-load inserter picks a single
    table for the whole program (one LoadActFuncSet instead of ~300)."""
    orig = _bacc_mod.get_activation_tables

    def pinned(arch):
        tabs = orig(arch)
        keep = "natural_log_exp_and_others"
        if keep in tabs:
            tabs = {k: (v if k == keep else set()) for k, v in tabs.items()}
        return tabs

    _bacc_mod.get_activation_tables = pinned
    try:
        yield
    finally:
        _bacc_mod.get_activation_tables = orig


BF16 = ml_dtypes.bfloat16
F8 = ml_dtypes.float8_e4m3
dt = mybir.dt
P = 128
N_CORES = 8
TBL_DT = dt.float8e4          # gather-table / message / M dtype


# ----------------------------------------------------------------------------
# Host-side edge preprocessing
# ----------------------------------------------------------------------------

def preprocess_edges(edge_src, edge_dst, edge_w, N, S):
    """Group edges per (dst 128-tile, src half), sorted by dst within each
    group.  Chunk = 128 gather slots; slot k*128+p holds sorted edge k*128+p.

    Emission plan (shared across cores):
      per (tile, half): for each pair of chunks j -> one DoubleRow matmul if
      the pair's edges lie in one 64-dst window on every core ("pure"), else
      two window-masked DoubleRow matmuls; a trailing odd chunk -> one plain
      [128,128] matmul.

    Returns (plan, M_list, idxl_list, idxh_list).
    """
    edge_src = np.asarray(edge_src).astype(np.int64)
    edge_dst = np.asarray(edge_dst).astype(np.int64)
    edge_w = np.asarray(edge_w, dtype=np.float32)
    n_tiles = (S + P - 1) // P
    HALF = N // 2

    per_core = []
    cnt = np.zeros((N_CORES, n_tiles, 2), dtype=np.int64)
    for c in range(N_CORES):
        sel = (edge_dst >= c * S) & (edge_dst < (c + 1) * S)
        dl = edge_dst[sel] - c * S
        sg = edge_src[sel]
        w = edge_w[sel]
        hi = (sg >= HALF).astype(np.int64)
        t = dl >> 7
        order = np.lexsort((dl, hi, t))
        dl, sg, w, hi, t = dl[order], sg[order], w[order], hi[order], t[order]
        gid = t * 2 + hi
        g = np.bincount(gid, minlength=2 * n_tiles)
        cnt[c] = g.reshape(n_tiles, 2)
        gstart = np.concatenate([[0], np.cumsum(g)])
        per_core.append((dl, sg, w, gid, gstart))

    cpw = np.maximum(1, (cnt.max(axis=0) + P - 1) // P)   # [n_tiles, 2]

    # ---- emission plan ----------------------------------------------------
    # blocks[t] = list of (h, kind, idx, w) in emission order; kind in
    # {"dr", "fat"}; idx = pair index j (dr) or chunk index k (fat);
    # w = 64-dst window (dr only; None for mixed covered via two entries).
    blocks = []
    nblk = np.zeros(n_tiles, dtype=np.int64)
    for t in range(n_tiles):
        bl = []
        for h in (0, 1):
            npair = int(cpw[t, h]) // 2
            odd = int(cpw[t, h]) % 2
            for j in range(npair):
                # pure if, on every core, all real edges of pair j fall in
                # one 64-window
                wset = set()
                for c in range(N_CORES):
                    dl, sg, w_, gid, gstart = per_core[c]
                    g0 = gstart[2 * t + h]
                    n = cnt[c, t, h]
                    a = min(256 * j, n)
                    b = min(256 * (j + 1), n)
                    if b > a:
                        dloc = dl[g0 + a:g0 + b] - 128 * t
                        if (dloc < 64).any():
                            wset.add(0)
                        if (dloc >= 64).any():
                            wset.add(1)
                if len(wset) <= 1:
                    bl.append((h, "dr", j, wset.pop() if wset else 0))
                else:
                    bl.append((h, "dr", j, 0))
                    bl.append((h, "dr", j, 1))
            if odd:
                bl.append((h, "fat", int(cpw[t, h]) - 1, None))
        blocks.append(bl)
        nblk[t] = len(bl)
    mblk_base = np.concatenate([[0], np.cumsum(nblk)])
    NBLK = int(mblk_base[-1])

    clo_base = np.concatenate([[0], np.cumsum(cpw[:, 0])])
    chi_base = np.concatenate([[0], np.cumsum(cpw[:, 1])])
    CLO, CHI = int(clo_base[-1]), int(chi_base[-1])

    # ---- per-core M / idx tensors ----------------------------------------
    M_list, idxl_list, idxh_list = [], [], []
    for c in range(N_CORES):
        dl, sg, w_, gid, gstart = per_core[c]
        M = np.zeros((P, NBLK * P), dtype=F8)
        idxl = np.zeros((P, CLO * 8), dtype=np.int16)
        idxh = np.zeros((P, CHI * 8), dtype=np.int16)
        for t in range(n_tiles):
            for bi, (h, kind, idx, wwin) in enumerate(blocks[t]):
                g0 = gstart[2 * t + h]
                n = int(cnt[c, t, h])
                col0 = (int(mblk_base[t]) + bi) * P
                if kind == "dr":
                    a = min(256 * idx, n)
                    b = min(256 * (idx + 1), n)
                    if b <= a:
                        continue
                    r = np.arange(a, b)
                    dloc = dl[g0 + a:g0 + b] - 128 * t
                    sel = (dloc >= 64) == (wwin == 1)
                    r, dloc = r[sel], dloc[sel]
                    i = (r - 256 * idx) >> 7
                    p = r & 127
                    M[p, col0 + i * 64 + (dloc - 64 * wwin)] = \
                        w_[g0 + r].astype(F8)
                else:  # fat
                    a = min(128 * idx, n)
                    b = min(128 * (idx + 1), n)
                    if b <= a:
                        continue
                    r = np.arange(a, b)
                    dloc = dl[g0 + a:g0 + b] - 128 * t
                    M[r & 127, col0 + dloc] = w_[g0 + r].astype(F8)
            # idx arrays: chunk k slot p -> sorted edge k*128+p (pad -> 0)
            for h, arr, base, off in ((0, idxl, clo_base, 0),
                                      (1, idxh, chi_base, HALF)):
                g0 = gstart[2 * t + h]
                n = int(cnt[c, t, h])
                nck = int(cpw[t, h])
                vals = np.zeros(nck * P, dtype=np.int16)
                vals[:n] = (sg[g0:g0 + n] - off).astype(np.int16)
                # wrapped layout: slot s -> row s%16 (replicated x8), col s//16
                cols = int(base[t]) * 8 + (np.arange(nck * P) >> 4)
                rows = np.arange(nck * P) & 15
                for g in range(8):
                    arr[16 * g + rows, cols] = vals
        M_list.append(M)
        idxl_list.append(idxl)
        idxh_list.append(idxh)

    plan = {
        "cpw": cpw, "blocks": blocks, "nblk": nblk, "mblk_base": mblk_base,
        "NBLK": NBLK, "clo_base": clo_base, "chi_base": chi_base,
        "CLO": CLO, "CHI": CHI, "n_tiles": n_tiles,
    }
    return plan, M_list, idxl_list, idxh_list


# ----------------------------------------------------------------------------
# Bass program
# ----------------------------------------------------------------------------

def build_nc(N, F0, H, C, S, plan, single_core=False):
    n_tiles = plan["n_tiles"]
    cpw = plan["cpw"]
    blocks = plan["blocks"]
    mblk_base = plan["mblk_base"]
    NBLK = plan["NBLK"]
    clo_base = plan["clo_base"]
    chi_base = plan["chi_base"]
    CLO, CHI = plan["CLO"], plan["CHI"]
    HALF = N // 2
    KF = F0 // P       # k-chunks of F0 (4)
    KH = H // P        # k-chunks of H (2)
    H2 = 2 * H
    DR = mybir.MatmulPerfMode.DoubleRow

    nc = bacc.Bacc("TRN2", num_devices=1 if single_core else N_CORES,
                   dynamic_dma_scratch_size=36864)

    # --- I/O ---
    x0T = nc.declare_dram_parameter("x0T", [F0, S], dt.bfloat16, isOutput=False)
    x1T = nc.declare_dram_parameter("x1T", [F0, S], dt.bfloat16, isOutput=False)
    W1a = nc.declare_dram_parameter("W1a", [F0, H], dt.bfloat16, isOutput=False)
    W1b = nc.declare_dram_parameter("W1b", [F0, H], dt.bfloat16, isOutput=False)
    W2a = nc.declare_dram_parameter("W2a", [H, H], dt.bfloat16, isOutput=False)
    W2b = nc.declare_dram_parameter("W2b", [H, H], dt.bfloat16, isOutput=False)
    LWa = nc.declare_dram_parameter("LWa", [H, H], dt.bfloat16, isOutput=False)
    LWb = nc.declare_dram_parameter("LWb", [H, H], dt.bfloat16, isOutput=False)
    LWf = nc.declare_dram_parameter("LWf", [H2, C], dt.bfloat16, isOutput=False)
    b1 = nc.declare_dram_parameter("b1", [P, H2], dt.bfloat16, isOutput=False)
    b2 = nc.declare_dram_parameter("b2", [P, H2], dt.bfloat16, isOutput=False)
    lba = nc.declare_dram_parameter("lba", [P, H], dt.bfloat16, isOutput=False)
    lbb = nc.declare_dram_parameter("lbb", [P, H], dt.bfloat16, isOutput=False)
    lbf = nc.declare_dram_parameter("lbf", [P, C], dt.bfloat16, isOutput=False)
    Mt = nc.declare_dram_parameter("M", [P, NBLK * P], TBL_DT, isOutput=False)
    IDXL = nc.declare_dram_parameter("IDXL", [P, CLO * 8], dt.int16, isOutput=False)
    IDXH = nc.declare_dram_parameter("IDXH", [P, CHI * 8], dt.int16, isOutput=False)
    out_t = nc.declare_dram_parameter("out", [S, C], dt.float32, isOutput=True)

    # --- internal DRAM ---
    u_loc = nc.dram_tensor("u_loc", [S, H2], TBL_DT)
    v_loc = nc.dram_tensor("v_loc", [S, H2], TBL_DT)
    if single_core:
        U = nc.declare_dram_parameter("Uin", [N, H2], TBL_DT, isOutput=False)
        V = nc.declare_dram_parameter("Vin", [N, H2], TBL_DT, isOutput=False)
    else:
        U = nc.dram_tensor("U", [N, H2], TBL_DT, addr_space="Shared")
        V = nc.dram_tensor("V", [N, H2], TBL_DT, addr_space="Shared")
    groups = [list(range(N_CORES))]

    with TileContext(nc, num_cores=N_CORES) as tc:
        ctx = contextlib.ExitStack()
        with ctx:
            perm = ctx.enter_context(tc.tile_pool(name="perm", bufs=1))
            big = ctx.enter_context(tc.tile_pool(name="big", bufs=1))
            mpool = ctx.enter_context(tc.tile_pool(name="mpool", bufs=2))
            msgp = ctx.enter_context(tc.tile_pool(name="msgp", bufs=2))
            sb = ctx.enter_context(tc.tile_pool(name="sb", bufs=2))
            stat = ctx.enter_context(tc.tile_pool(name="stat", bufs=4))
            ps_big = ctx.enter_context(tc.tile_pool(name="ps_big", bufs=2, space="PSUM"))
            ps_d = ctx.enter_context(tc.tile_pool(name="ps_d", bufs=2, space="PSUM"))
            ps_t = ctx.enter_context(tc.tile_pool(name="ps_t", bufs=2, space="PSUM"))
            ps_f = ctx.enter_context(tc.tile_pool(name="ps_f", bufs=2, space="PSUM"))

            # persistent small tiles
            ident = perm.tile([P, P], dt.bfloat16, tag="ident")
            make_identity(nc, ident[:])
            ones_t = perm.tile([P, P], dt.bfloat16, tag="ones")
            nc.vector.memset(ones_t[:], 1.0)
            w1a_t = [perm.tile([P, H], dt.bfloat16, name=f"w1a{k}", tag=f"w1a{k}") for k in range(KF)]
            w1b_t = [perm.tile([P, H], dt.bfloat16, name=f"w1b{k}", tag=f"w1b{k}") for k in range(KF)]
            w2a_t = [perm.tile([P, H], dt.bfloat16, name=f"w2a{k}", tag=f"w2a{k}") for k in range(KH)]
            w2b_t = [perm.tile([P, H], dt.bfloat16, name=f"w2b{k}", tag=f"w2b{k}") for k in range(KH)]
            lwa_t = [perm.tile([P, H], dt.bfloat16, name=f"lwa{k}", tag=f"lwa{k}") for k in range(KH)]
            lwb_t = [perm.tile([P, H], dt.bfloat16, name=f"lwb{k}", tag=f"lwb{k}") for k in range(KH)]
            lwf_t = [perm.tile([P, C], dt.bfloat16, name=f"lwf{k}", tag=f"lwf{k}") for k in range(2 * KH)]
            for k in range(KF):
                nc.sync.dma_start(out=w1a_t[k][:], in_=W1a[k * P:(k + 1) * P, :])
                nc.sync.dma_start(out=w1b_t[k][:], in_=W1b[k * P:(k + 1) * P, :])
            for k in range(KH):
                nc.sync.dma_start(out=w2a_t[k][:], in_=W2a[k * P:(k + 1) * P, :])
                nc.sync.dma_start(out=w2b_t[k][:], in_=W2b[k * P:(k + 1) * P, :])
                nc.sync.dma_start(out=lwa_t[k][:], in_=LWa[k * P:(k + 1) * P, :])
                nc.sync.dma_start(out=lwb_t[k][:], in_=LWb[k * P:(k + 1) * P, :])
            for k in range(2 * KH):
                nc.sync.dma_start(out=lwf_t[k][:], in_=LWf[k * P:(k + 1) * P, :])
            b1_t = perm.tile([P, H2], dt.bfloat16, tag="b1")
            b2_t = perm.tile([P, H2], dt.bfloat16, tag="b2")
            lba_t = perm.tile([P, H], dt.bfloat16, tag="lba")
            lbb_t = perm.tile([P, H], dt.bfloat16, tag="lbb")
            lbf_t = perm.tile([P, C], dt.bfloat16, tag="lbf")
            nc.sync.dma_start(out=b1_t[:], in_=b1[:])
            nc.sync.dma_start(out=b2_t[:], in_=b2[:])
            nc.sync.dma_start(out=lba_t[:], in_=lba[:])
            nc.sync.dma_start(out=lbb_t[:], in_=lbb[:])
            nc.sync.dma_start(out=lbf_t[:], in_=lbf[:])
            idxl_t = perm.tile([P, CLO * 8], dt.int16, tag="idxl")
            nc.sync.dma_start(out=idxl_t[:], in_=IDXL[:])
            idxh_t = perm.tile([P, CHI * 8], dt.int16, tag="idxh")
            nc.sync.dma_start(out=idxh_t[:], in_=IDXH[:])

            # two big feature-major tiles [P, 4S]; reused across phases:
            #   phase A in: big0 = x0T (4 k-chunks), big1 = x1T
            #   phase C out: big0 = hT (ha0 ha1 hb0 hb1 chunk-major)
            #   phase F out: big1 = gT
            #   phase G out: big0 = zT
            big0 = big.tile([P, 4 * S], dt.bfloat16, tag="big0")
            big1 = big.tile([P, 4 * S], dt.bfloat16, tag="big1")
            NQ = 4   # load x in column quarters so phase A starts early
            qb = [0] + [((q + 1) * S // NQ + P - 1) // P * P for q in range(NQ - 1)] + [S]
            for q in range(NQ):
                a, b = qb[q], qb[q + 1]
                for k in range(KF):
                    nc.sync.dma_start(out=big0[:, k * S + a:k * S + b],
                                      in_=x0T[k * P:(k + 1) * P, a:b])
                for k in range(KF):
                    nc.sync.dma_start(out=big1[:, k * S + a:k * S + b],
                                      in_=x1T[k * P:(k + 1) * P, a:b])

            def mtile(m):
                ms = m * P
                return ms, min(P, S - ms)

            # ---------------- Phase A: u = x @ W1 (both branches) ----------
            for m in range(n_tiles):
                ms, mw = mtile(m)
                pa = ps_d.tile([P, H], dt.float32, tag="ps_d")
                pb = ps_d.tile([P, H], dt.float32, tag="ps_d")
                for k in range(KF):
                    nc.tensor.matmul(pa[:mw, :], lhsT=big0[:, k * S + ms:k * S + ms + mw],
                                     rhs=w1a_t[k][:], start=(k == 0), stop=(k == KF - 1))
                for k in range(KF):
                    nc.tensor.matmul(pb[:mw, :], lhsT=big1[:, k * S + ms:k * S + ms + mw],
                                     rhs=w1b_t[k][:], start=(k == 0), stop=(k == KF - 1))
                uab = sb.tile([P, H2], TBL_DT, tag="uab")
                nc.scalar.activation(out=uab[:mw, :H], in_=pa[:mw, :],
                                     func=mybir.ActivationFunctionType.Copy)
                nc.scalar.activation(out=uab[:mw, H:], in_=pb[:mw, :],
                                     func=mybir.ActivationFunctionType.Copy)
                nc.sync.dma_start(out=u_loc[ms:ms + mw, :], in_=uab[:mw, :])

            # ---------------- Phase B: AllGather u ------------------------
            if not single_core:
                nc.gpsimd.collective_compute(
                    "AllGather", mybir.AluOpType.bypass, replica_groups=groups,
                    ins=[u_loc[:]], outs=[U[:]])

            # ---------------- spmm tile emitter ---------------------------
            def spmm_tile(t, table, bias_t, relu, outT):
                """One dst tile: gather + aggregate + bias/act + transpose
                into feature-major outT [P, 4S]."""
                ts_, tw = mtile(t)
                nlo, nhi = int(cpw[t, 0]), int(cpw[t, 1])
                nch = nlo + nhi
                bl = blocks[t]
                nb = len(bl)
                mb0 = int(mblk_base[t])
                ph = ps_big.tile([P, H2], dt.float32, tag="ps_big")
                mt = mpool.tile([P, nb * P], TBL_DT, tag="mt")
                nc.sync.dma_start(out=mt[:], in_=Mt[:, mb0 * P:(mb0 + nb) * P])
                msg = msgp.tile([P, nch * H2], TBL_DT, tag="msg")
                for h, n_k, base, it in ((0, nlo, clo_base, idxl_t),
                                         (1, nhi, chi_base, idxh_t)):
                    o = int(base[t]) * 8
                    co = 0 if h == 0 else nlo
                    nc.gpsimd.dma_gather(
                        out_ap=msg[:, co * H2:(co + n_k) * H2].rearrange(
                            "p (n e) -> p n e", e=H2),
                        in_ap=table[:HALF, :] if h == 0 else table[HALF:, :],
                        idxs_ap=it[:, o:o + n_k * 8],
                        num_idxs=n_k * P, num_idxs_reg=n_k * P,
                        elem_size=H2)
                # bias opener (start=True zeroes the whole tile's psum)
                nc.tensor.matmul(ph[:, :], lhsT=ones_t[0:1, :],
                                 rhs=bias_t[0:1, :], start=True, stop=False,
                                 skip_group_check=True)
                for bi, (h, kind, idx, wwin) in enumerate(bl):
                    co = 0 if h == 0 else nlo
                    last = (bi == nb - 1)
                    if kind == "dr":
                        nc.tensor.matmul(
                            ph[64 * wwin:64 * wwin + 64, :],
                            lhsT=mt[:, bi * P:(bi + 1) * P].rearrange(
                                "p (i d) -> p i d", i=2),
                            rhs=msg[:, (co + 2 * idx) * H2:(co + 2 * idx + 2) * H2
                                    ].rearrange("p (i e) -> p i e", i=2),
                            start=False, stop=last, perf_mode=DR,
                            skip_group_check=True)
                    else:
                        nc.tensor.matmul(
                            ph[:, :],
                            lhsT=mt[:, bi * P:(bi + 1) * P],
                            rhs=msg[:, (co + idx) * H2:(co + idx + 1) * H2],
                            start=False, stop=last,
                            skip_group_check=True)
                hab = sb.tile([P, H2], dt.bfloat16, tag="hab")
                nc.scalar.activation(
                    out=hab[:tw, :], in_=ph[:tw, :],
                    func=(mybir.ActivationFunctionType.Relu if relu
                          else mybir.ActivationFunctionType.Copy))
                pt = ps_t.tile([P, H2], dt.bfloat16, tag="ps_t")
                for fc in range(2 * KH):
                    nc.tensor.transpose(out=pt[:, fc * P:fc * P + tw],
                                        in_=hab[:tw, fc * P:(fc + 1) * P],
                                        identity=ident[:tw, :tw])
                nc.scalar.activation(
                    out=outT[:, :].rearrange("p (f s) -> p f s", f=4)[:, :, ts_:ts_ + tw],
                    in_=pt[:, :].rearrange("p (f s) -> p f s", f=4)[:, :, :tw],
                    func=mybir.ActivationFunctionType.Copy)

            # -------- Phases C+D fused per tile: h = relu(spmm(U) + b1);
            # -------- v = h @ W2 ------------------------------------------
            for m in range(n_tiles):
                ms, mw = mtile(m)
                spmm_tile(m, U, b1_t, True, big0)
                pa = ps_d.tile([P, H], dt.float32, tag="ps_d")
                pb = ps_d.tile([P, H], dt.float32, tag="ps_d")
                for k in range(KH):
                    nc.tensor.matmul(pa[:mw, :], lhsT=big0[:, k * S + ms:k * S + ms + mw],
                                     rhs=w2a_t[k][:], start=(k == 0), stop=(k == KH - 1))
                for k in range(KH):
                    nc.tensor.matmul(pb[:mw, :],
                                     lhsT=big0[:, (KH + k) * S + ms:(KH + k) * S + ms + mw],
                                     rhs=w2b_t[k][:], start=(k == 0), stop=(k == KH - 1))
                vab = sb.tile([P, H2], TBL_DT, tag="vab")
                nc.scalar.activation(out=vab[:mw, :H], in_=pa[:mw, :],
                                     func=mybir.ActivationFunctionType.Copy)
                nc.scalar.activation(out=vab[:mw, H:], in_=pb[:mw, :],
                                     func=mybir.ActivationFunctionType.Copy)
                nc.sync.dma_start(out=v_loc[ms:ms + mw, :], in_=vab[:mw, :])

            # ---------------- Phase E: AllGather v ------------------------
            if not single_core:
                nc.gpsimd.collective_compute(
                    "AllGather", mybir.AluOpType.bypass, replica_groups=groups,
                    ins=[v_loc[:]], outs=[V[:]])

            # ---- Phases F+G+H fused per tile -----------------------------
            def softmax_z(py, lb_t, zdst, mw, width):
                """zdst <- log_softmax(py + lb) ; py is PSUM [P, width] f32."""
                yf = sb.tile([P, width], dt.float32, tag=f"yf{width}")
                nc.vector.tensor_tensor(out=yf[:mw, :], in0=py[:mw, :],
                                        in1=lb_t[:mw, :], op=mybir.AluOpType.add)
                nmx = stat.tile([P, 1], dt.float32, tag="nmx")
                nc.vector.tensor_reduce(out=nmx[:mw, :], in_=yf[:mw, :],
                                        axis=mybir.AxisListType.X,
                                        op=mybir.AluOpType.max, negate=True)
                ex = sb.tile([P, width], dt.float32, tag=f"ex{width}")
                sx = stat.tile([P, 1], dt.float32, tag="sx")
                nc.scalar.activation(out=ex[:mw, :], in_=yf[:mw, :],
                                     func=mybir.ActivationFunctionType.Exp,
                                     bias=nmx[:mw, :], scale=1.0,
                                     accum_out=sx[:mw, :])
                lse = stat.tile([P, 1], dt.float32, tag="lse")
                nc.scalar.activation(out=lse[:mw, :], in_=sx[:mw, :],
                                     func=mybir.ActivationFunctionType.Ln)
                nc.vector.tensor_scalar(out=zdst, in0=yf[:mw, :],
                                        scalar1=nmx[:mw, :], scalar2=lse[:mw, :],
                                        op0=mybir.AluOpType.add,
                                        op1=mybir.AluOpType.subtract)

            for m in range(n_tiles):
                ms, mw = mtile(m)
                # F: g = spmm(V) + b2 -> big1 feature-major
                spmm_tile(m, V, b2_t, False, big1)
                # G: z = log_softmax(g @ LW + Lb) -> big0 feature-major
                zab = sb.tile([P, H2], dt.bfloat16, tag="zab")
                for br, (lw_t, lb_t) in enumerate(
                        ((lwa_t, lba_t), (lwb_t, lbb_t))):
                    py = ps_d.tile([P, H], dt.float32, tag="ps_d")
                    for k in range(KH):
                        nc.tensor.matmul(
                            py[:mw, :],
                            lhsT=big1[:, (2 * br + k) * S + ms:(2 * br + k) * S + ms + mw],
                            rhs=lw_t[k][:], start=(k == 0), stop=(k == KH - 1))
                    softmax_z(py, lb_t, zab[:mw, br * H:(br + 1) * H], mw, H)
                pt = ps_t.tile([P, H2], dt.bfloat16, tag="ps_t")
                for fc in range(2 * KH):
                    nc.tensor.transpose(out=pt[:, fc * P:fc * P + mw],
                                        in_=zab[:mw, fc * P:(fc + 1) * P],
                                        identity=ident[:mw, :mw])
                nc.scalar.activation(
                    out=big0[:, :].rearrange("p (f s) -> p f s", f=4)[:, :, ms:ms + mw],
                    in_=pt[:, :].rearrange("p (f s) -> p f s", f=4)[:, :, :mw],
                    func=mybir.ActivationFunctionType.Copy)
                # H: out = log_softmax(z @ LWf + Lb)
                pf = ps_f.tile([P, C], dt.float32, tag="ps_f")
                for k in range(2 * KH):
                    nc.tensor.matmul(pf[:mw, :],
                                     lhsT=big0[:, k * S + ms:k * S + ms + mw],
                                     rhs=lwf_t[k][:], start=(k == 0),
                                     stop=(k == 2 * KH - 1))
                ot = sb.tile([P, C], dt.float32, tag="ot")
                softmax_z(pf, lbf_t, ot[:mw, :], mw, C)
                nc.sync.dma_start(out=out_t[ms:ms + mw, :], in_=ot[:mw, :])

    import os
    if os.environ.get("NO_ACT_PIN"):
        nc.compile()
    else:
        with _pinned_act_tables():
            nc.compile()
    return nc


# ----------------------------------------------------------------------------
# Entry point
# ----------------------------------------------------------------------------

_CACHE = {}


def kernel(x0, x1, edge_src, edge_dst, edge_w,
           W1a, b1a, W2a, b2a, LWa, Lba,
           W1b, b1b, W2b, b2b, LWb, Lbb,
           LW, Lb):
    x0 = np.asarray(x0)
    x1 = np.asarray(x1)
    N, F0 = x0.shape
    H = np.asarray(W1a).shape[1]
    C = np.asarray(LW).shape[1]
    S = N // N_CORES

    key = (N, F0, H, C,
           hash(np.asarray(edge_src).tobytes()) ^ hash(np.asarray(edge_dst).tobytes()))
    if key not in _CACHE:
        plan, M_list, idxl_list, idxh_list = preprocess_edges(
            edge_src, edge_dst, edge_w, N, S)
        nc = build_nc(N, F0, H, C, S, plan)
        _CACHE[key] = (nc, M_list, idxl_list, idxh_list)
    nc, M_list, idxl_list, idxh_list = _CACHE[key]

    bf = lambda a: np.asarray(a, dtype=BF16)
    f32 = lambda a: np.asarray(a, dtype=np.float32)
    bcast = lambda v: np.broadcast_to(np.asarray(v, dtype=BF16)[None, :], (P, len(v))).copy()

    x0T = bf(x0).T
    x1T = bf(x1).T
    shared = {
        "W1a": bf(W1a), "W1b": bf(W1b), "W2a": bf(W2a), "W2b": bf(W2b),
        "LWa": bf(LWa), "LWb": bf(LWb), "LWf": bf(LW),
        "b1": bcast(np.concatenate([f32(b1a), f32(b1b)])),
        "b2": bcast(np.concatenate([f32(b2a), f32(b2b)])),
        "lba": bcast(f32(Lba)), "lbb": bcast(f32(Lbb)), "lbf": bcast(f32(Lb)),
    }
    in_maps = []
    for c in range(N_CORES):
        in_maps.append({
            **shared,
            "x0T": np.ascontiguousarray(x0T[:, c * S:(c + 1) * S]),
            "x1T": np.ascontiguousarray(x1T[:, c * S:(c + 1) * S]),
            "M": M_list[c], "IDXL": idxl_list[c], "IDXH": idxh_list[c],
        })
    res = run_bass_kernel_spmd(nc, in_maps, list(range(N_CORES)))
    return np.concatenate([res.results[c]["out"] for c in range(N_CORES)], axis=0)


# revision 11
# speedup vs baseline: 1.9689x; 1.0486x over previous
"""Bass/Trainium2 kernel for the 2-branch GCN (gnn_message_passing).

Computation (reference):
    per branch i in {a, b}:
        u_i = x_i @ W1_i                                  [N, H]
        h_i = relu(spmm(A, u_i) + b1_i)                   [N, H]
        v_i = h_i @ W2_i                                  [N, H]
        g_i = spmm(A, v_i) + b2_i                         [N, H]
        z_i = log_softmax(g_i @ LW_i + Lb_i)              [N, H]
    out = log_softmax(concat(z_a, z_b) @ LW + Lb)         [N, C]
where spmm(A, u)[d] = sum_{e: dst[e]=d} w[e] * u[src[e]].

Strategy (8 NeuronCores, node-sharded, fp8 message path):
  - Core c owns node rows [c*S, (c+1)*S), S = N/8.  Dense matmuls in bf16.
  - Activation tables U = allgather(x@W1), V = allgather(h@W2) stored fp8e4
    (concat a|b features -> 512B rows); both spmm layers gather rows of the
    concat table once per edge (512B descriptors, the 1x-latency minimum).
  - Edges grouped per (dst 128-tile, src half); chunk counts are padded to
    the max across the 8 cores so the compiled program is shared (SPMD).
  - Aggregation: one-hot matrices M (fp8, edge weight at the dst column)
    multiply gathered messages on the PE.  Chunk pairs whose 256 edges fall
    in one 64-dst window on ALL cores use a single DoubleRow fp8 matmul
    (0.5 cycles/row); mixed pairs emit two window-masked DoubleRow matmuls;
    a trailing odd chunk uses a plain [128,128] fp8 matmul.
  - Bias rides a K=1 matmul (ones x bias row) that also opens (start=True)
    each 64-row PSUM region; relu/cast psum->SBUF is one ACT op.
  - Feature-major activations live in two [128, 4S] SBUF tiles (h, g, z
    reuse the x0/x1 space); writeback per tile = 4 PE transposes into one
    PSUM bank + one 4-block strided ACT copy.
"""

import sys

if "/opt/trn_rl_repo" not in sys.path:
    sys.path.insert(0, "/opt/trn_rl_repo")

import numpy as np
import ml_dtypes

import concourse.bass as bass
import concourse.bacc as bacc
import concourse.mybir as mybir
import concourse.tile as tile
from concourse.tile import TileContext
from concourse.masks import make_identity
from concourse.bass_utils import run_bass_kernel_spmd

import contextlib
import concourse.bacc as _bacc_mod


@contextlib.contextmanager
def _pinned_act_tables():
    """During compile, make every activation-function table except the
    all-purpose one look empty so bacc's table-load inserter picks a single
    table for the whole program (one LoadActFuncSet instead of ~300)."""
    orig = _bacc_mod.get_activation_tables

    def pinned(arch):
        tabs = orig(arch)
        keep = "natural_log_exp_and_others"
        if keep in tabs:
            tabs = {k: (v if k == keep else set()) for k, v in tabs.items()}
        return tabs

    _bacc_mod.get_activation_tables = pinned
    try:
        yield
    finally:
        _bacc_mod.get_activation_tables = orig


BF16 = ml_dtypes.bfloat16
F8 = ml_dtypes.float8_e4m3
dt = mybir.dt
P = 128
N_CORES = 8
TBL_DT = dt.float8e4          # gather-table / message / M dtype


# ----------------------------------------------------------------------------
# Host-side edge preprocessing
# ----------------------------------------------------------------------------

def preprocess_edges(edge_src, edge_dst, edge_w, N, S):
    """Group edges per (dst 128-tile, src half), sorted by dst within each
    group.  Chunk = 128 gather slots; slot k*128+p holds sorted edge k*128+p.

    Emission plan (shared across cores):
      per (tile, half): for each pair of chunks j -> one DoubleRow matmul if
      the pair's edges lie in one 64-dst window on every core ("pure"), else
      two window-masked DoubleRow matmuls; a trailing odd chunk -> one plain
      [128,128] matmul.

    Returns (plan, M_list, idxl_list, idxh_list).
    """
    edge_src = np.asarray(edge_src).astype(np.int64)
    edge_dst = np.asarray(edge_dst).astype(np.int64)
    edge_w = np.asarray(edge_w, dtype=np.float32)
    n_tiles = (S + P - 1) // P
    HALF = N // 2

    per_core = []
    cnt = np.zeros((N_CORES, n_tiles, 2), dtype=np.int64)
    for c in range(N_CORES):
        sel = (edge_dst >= c * S) & (edge_dst < (c + 1) * S)
        dl = edge_dst[sel] - c * S
        sg = edge_src[sel]
        w = edge_w[sel]
        hi = (sg >= HALF).astype(np.int64)
        t = dl >> 7
        order = np.lexsort((dl, hi, t))
        dl, sg, w, hi, t = dl[order], sg[order], w[order], hi[order], t[order]
        gid = t * 2 + hi
        g = np.bincount(gid, minlength=2 * n_tiles)
        cnt[c] = g.reshape(n_tiles, 2)
        gstart = np.concatenate([[0], np.cumsum(g)])
        per_core.append((dl, sg, w, gid, gstart))

    cpw = np.maximum(1, (cnt.max(axis=0) + P - 1) // P)   # [n_tiles, 2]

    # ---- emission plan ----------------------------------------------------
    # blocks[t] = list of (h, kind, idx, w) in emission order; kind in
    # {"dr", "fat"}; idx = pair index j (dr) or chunk index k (fat);
    # w = 64-dst window (dr only; None for mixed covered via two entries).
    blocks = []
    nblk = np.zeros(n_tiles, dtype=np.int64)
    for t in range(n_tiles):
        bl = []
        for h in (0, 1):
            npair = int(cpw[t, h]) // 2
            odd = int(cpw[t, h]) % 2
            for j in range(npair):
                # pure if, on every core, all real edges of pair j fall in
                # one 64-window
                wset = set()
                for c in range(N_CORES):
                    dl, sg, w_, gid, gstart = per_core[c]
                    g0 = gstart[2 * t + h]
                    n = cnt[c, t, h]
                    a = min(256 * j, n)
                    b = min(256 * (j + 1), n)
                    if b > a:
                        dloc = dl[g0 + a:g0 + b] - 128 * t
                        if (dloc < 64).any():
                            wset.add(0)
                        if (dloc >= 64).any():
                            wset.add(1)
                if len(wset) <= 1:
                    bl.append((h, "dr", j, wset.pop() if wset else 0))
                else:
                    bl.append((h, "dr", j, 0))
                    bl.append((h, "dr", j, 1))
            if odd:
                bl.append((h, "fat", int(cpw[t, h]) - 1, None))
        blocks.append(bl)
        nblk[t] = len(bl)
    mblk_base = np.concatenate([[0], np.cumsum(nblk)])
    NBLK = int(mblk_base[-1])

    clo_base = np.concatenate([[0], np.cumsum(cpw[:, 0])])
    chi_base = np.concatenate([[0], np.cumsum(cpw[:, 1])])
    CLO, CHI = int(clo_base[-1]), int(chi_base[-1])

    # ---- per-core M / idx tensors ----------------------------------------
    M_list, idxl_list, idxh_list = [], [], []
    for c in range(N_CORES):
        dl, sg, w_, gid, gstart = per_core[c]
        M = np.zeros((P, NBLK * P), dtype=F8)
        idxl = np.zeros((P, CLO * 8), dtype=np.int16)
        idxh = np.zeros((P, CHI * 8), dtype=np.int16)
        for t in range(n_tiles):
            for bi, (h, kind, idx, wwin) in enumerate(blocks[t]):
                g0 = gstart[2 * t + h]
                n = int(cnt[c, t, h])
                col0 = (int(mblk_base[t]) + bi) * P
                if kind == "dr":
                    a = min(256 * idx, n)
                    b = min(256 * (idx + 1), n)
                    if b <= a:
                        continue
                    r = np.arange(a, b)
                    dloc = dl[g0 + a:g0 + b] - 128 * t
                    sel = (dloc >= 64) == (wwin == 1)
                    r, dloc = r[sel], dloc[sel]
                    i = (r - 256 * idx) >> 7
                    p = r & 127
                    M[p, col0 + i * 64 + (dloc - 64 * wwin)] = \
                        w_[g0 + r].astype(F8)
                else:  # fat
                    a = min(128 * idx, n)
                    b = min(128 * (idx + 1), n)
                    if b <= a:
                        continue
                    r = np.arange(a, b)
                    dloc = dl[g0 + a:g0 + b] - 128 * t
                    M[r & 127, col0 + dloc] = w_[g0 + r].astype(F8)
            # idx arrays: chunk k slot p -> sorted edge k*128+p (pad -> 0)
            for h, arr, base, off in ((0, idxl, clo_base, 0),
                                      (1, idxh, chi_base, HALF)):
                g0 = gstart[2 * t + h]
                n = int(cnt[c, t, h])
                nck = int(cpw[t, h])
                vals = np.zeros(nck * P, dtype=np.int16)
                vals[:n] = (sg[g0:g0 + n] - off).astype(np.int16)
                # wrapped layout: slot s -> row s%16 (replicated x8), col s//16
                cols = int(base[t]) * 8 + (np.arange(nck * P) >> 4)
                rows = np.arange(nck * P) & 15
                for g in range(8):
                    arr[16 * g + rows, cols] = vals
        M_list.append(M)
        idxl_list.append(idxl)
        idxh_list.append(idxh)

    plan = {
        "cpw": cpw, "blocks": blocks, "nblk": nblk, "mblk_base": mblk_base,
        "NBLK": NBLK, "clo_base": clo_base, "chi_base": chi_base,
        "CLO": CLO, "CHI": CHI, "n_tiles": n_tiles,
    }
    return plan, M_list, idxl_list, idxh_list


# ----------------------------------------------------------------------------
# Bass program
# ----------------------------------------------------------------------------

def build_nc(N, F0, H, C, S, plan, single_core=False):
    n_tiles = plan["n_tiles"]
    cpw = plan["cpw"]
    blocks = plan["blocks"]
    mblk_base = plan["mblk_base"]
    NBLK = plan["NBLK"]
    clo_base = plan["clo_base"]
    chi_base = plan["chi_base"]
    CLO, CHI = plan["CLO"], plan["CHI"]
    HALF = N // 2
    KF = F0 // P       # k-chunks of F0 (4)
    KH = H // P        # k-chunks of H (2)
    H2 = 2 * H
    DR = mybir.MatmulPerfMode.DoubleRow

    nc = bacc.Bacc("TRN2", num_devices=1 if single_core else N_CORES,
                   dynamic_dma_scratch_size=36864)

    # --- I/O ---
    x0T = nc.declare_dram_parameter("x0T", [F0, S], dt.bfloat16, isOutput=False)
    x1T = nc.declare_dram_parameter("x1T", [F0, S], dt.bfloat16, isOutput=False)
    W1a = nc.declare_dram_parameter("W1a", [F0, H], dt.bfloat16, isOutput=False)
    W1b = nc.declare_dram_parameter("W1b", [F0, H], dt.bfloat16, isOutput=False)
    W2a = nc.declare_dram_parameter("W2a", [H, H], dt.bfloat16, isOutput=False)
    W2b = nc.declare_dram_parameter("W2b", [H, H], dt.bfloat16, isOutput=False)
    LWa = nc.declare_dram_parameter("LWa", [H, H], dt.bfloat16, isOutput=False)
    LWb = nc.declare_dram_parameter("LWb", [H, H], dt.bfloat16, isOutput=False)
    LWf = nc.declare_dram_parameter("LWf", [H2, C], dt.bfloat16, isOutput=False)
    b1 = nc.declare_dram_parameter("b1", [P, H2], dt.bfloat16, isOutput=False)
    b2 = nc.declare_dram_parameter("b2", [P, H2], dt.bfloat16, isOutput=False)
    lba = nc.declare_dram_parameter("lba", [P, H], dt.bfloat16, isOutput=False)
    lbb = nc.declare_dram_parameter("lbb", [P, H], dt.bfloat16, isOutput=False)
    lbf = nc.declare_dram_parameter("lbf", [P, C], dt.bfloat16, isOutput=False)
    Mt = nc.declare_dram_parameter("M", [P, NBLK * P], TBL_DT, isOutput=False)
    IDXL = nc.declare_dram_parameter("IDXL", [P, CLO * 8], dt.int16, isOutput=False)
    IDXH = nc.declare_dram_parameter("IDXH", [P, CHI * 8], dt.int16, isOutput=False)
    out_t = nc.declare_dram_parameter("out", [S, C], dt.float32, isOutput=True)

    # --- internal DRAM ---
    u_loc = nc.dram_tensor("u_loc", [S, H2], TBL_DT)
    v_loc = nc.dram_tensor("v_loc", [S, H2], TBL_DT)
    if single_core:
        U = nc.declare_dram_parameter("Uin", [N, H2], TBL_DT, isOutput=False)
        V = nc.declare_dram_parameter("Vin", [N, H2], TBL_DT, isOutput=False)
    else:
        U = nc.dram_tensor("U", [N, H2], TBL_DT, addr_space="Shared")
        V = nc.dram_tensor("V", [N, H2], TBL_DT, addr_space="Shared")
    groups = [list(range(N_CORES))]

    with TileContext(nc, num_cores=N_CORES) as tc:
        ctx = contextlib.ExitStack()
        with ctx:
            perm = ctx.enter_context(tc.tile_pool(name="perm", bufs=1))
            big = ctx.enter_context(tc.tile_pool(name="big", bufs=1))
            mpool = ctx.enter_context(tc.tile_pool(name="mpool", bufs=3))
            msgp = ctx.enter_context(tc.tile_pool(name="msgp", bufs=3))
            sb = ctx.enter_context(tc.tile_pool(name="sb", bufs=3))
            stat = ctx.enter_context(tc.tile_pool(name="stat", bufs=4))
            ps_big = ctx.enter_context(tc.tile_pool(name="ps_big", bufs=3, space="PSUM"))
            ps_d = ctx.enter_context(tc.tile_pool(name="ps_d", bufs=3, space="PSUM"))
            ps_t = ctx.enter_context(tc.tile_pool(name="ps_t", bufs=2, space="PSUM"))

            # persistent small tiles
            ident = perm.tile([P, P], dt.bfloat16, tag="ident")
            make_identity(nc, ident[:])
            ones_t = perm.tile([P, P], dt.bfloat16, tag="ones")
            nc.vector.memset(ones_t[:], 1.0)
            w1a_t = [perm.tile([P, H], dt.bfloat16, name=f"w1a{k}", tag=f"w1a{k}") for k in range(KF)]
            w1b_t = [perm.tile([P, H], dt.bfloat16, name=f"w1b{k}", tag=f"w1b{k}") for k in range(KF)]
            w2a_t = [perm.tile([P, H], dt.bfloat16, name=f"w2a{k}", tag=f"w2a{k}") for k in range(KH)]
            w2b_t = [perm.tile([P, H], dt.bfloat16, name=f"w2b{k}", tag=f"w2b{k}") for k in range(KH)]
            lwa_t = [perm.tile([P, H], dt.bfloat16, name=f"lwa{k}", tag=f"lwa{k}") for k in range(KH)]
            lwb_t = [perm.tile([P, H], dt.bfloat16, name=f"lwb{k}", tag=f"lwb{k}") for k in range(KH)]
            lwf_t = [perm.tile([P, C], dt.bfloat16, name=f"lwf{k}", tag=f"lwf{k}") for k in range(2 * KH)]
            for k in range(KF):
                nc.sync.dma_start(out=w1a_t[k][:], in_=W1a[k * P:(k + 1) * P, :])
                nc.sync.dma_start(out=w1b_t[k][:], in_=W1b[k * P:(k + 1) * P, :])
            for k in range(KH):
                nc.sync.dma_start(out=w2a_t[k][:], in_=W2a[k * P:(k + 1) * P, :])
                nc.sync.dma_start(out=w2b_t[k][:], in_=W2b[k * P:(k + 1) * P, :])
                nc.sync.dma_start(out=lwa_t[k][:], in_=LWa[k * P:(k + 1) * P, :])
                nc.sync.dma_start(out=lwb_t[k][:], in_=LWb[k * P:(k + 1) * P, :])
            for k in range(2 * KH):
                nc.sync.dma_start(out=lwf_t[k][:], in_=LWf[k * P:(k + 1) * P, :])
            b1_t = perm.tile([P, H2], dt.bfloat16, tag="b1")
            b2_t = perm.tile([P, H2], dt.bfloat16, tag="b2")
            lba_t = perm.tile([P, H], dt.bfloat16, tag="lba")
            lbb_t = perm.tile([P, H], dt.bfloat16, tag="lbb")
            lbf_t = perm.tile([P, C], dt.bfloat16, tag="lbf")
            nc.sync.dma_start(out=b1_t[:], in_=b1[:])
            nc.sync.dma_start(out=b2_t[:], in_=b2[:])
            nc.sync.dma_start(out=lba_t[:], in_=lba[:])
            nc.sync.dma_start(out=lbb_t[:], in_=lbb[:])
            nc.sync.dma_start(out=lbf_t[:], in_=lbf[:])
            idxl_t = perm.tile([P, CLO * 8], dt.int16, tag="idxl")
            nc.sync.dma_start(out=idxl_t[:], in_=IDXL[:])
            idxh_t = perm.tile([P, CHI * 8], dt.int16, tag="idxh")
            nc.sync.dma_start(out=idxh_t[:], in_=IDXH[:])

            # two big feature-major tiles [P, 4S]; reused across phases:
            #   phase A in: big0 = x0T (4 k-chunks), big1 = x1T
            #   phase C out: big0 = hT (ha0 ha1 hb0 hb1 chunk-major)
            #   phase F out: big1 = gT
            #   phase G out: big0 = zT
            big0 = big.tile([P, 4 * S], dt.bfloat16, tag="big0")
            big1 = big.tile([P, 4 * S], dt.bfloat16, tag="big1")
            NQ = 4   # load x in column quarters so phase A starts early
            qb = [0] + [((q + 1) * S // NQ + P - 1) // P * P for q in range(NQ - 1)] + [S]
            for q in range(NQ):
                a, b = qb[q], qb[q + 1]
                for k in range(KF):
                    nc.sync.dma_start(out=big0[:, k * S + a:k * S + b],
                                      in_=x0T[k * P:(k + 1) * P, a:b])
                for k in range(KF):
                    nc.sync.dma_start(out=big1[:, k * S + a:k * S + b],
                                      in_=x1T[k * P:(k + 1) * P, a:b])

            def mtile(m):
                ms = m * P
                return ms, min(P, S - ms)

            # ---------------- Phase A: u = x @ W1 (both branches) ----------
            for m in range(n_tiles):
                ms, mw = mtile(m)
                pa = ps_d.tile([P, H], dt.float32, tag="ps_d")
                pb = ps_d.tile([P, H], dt.float32, tag="ps_d")
                for k in range(KF):
                    nc.tensor.matmul(pa[:mw, :], lhsT=big0[:, k * S + ms:k * S + ms + mw],
                                     rhs=w1a_t[k][:], start=(k == 0), stop=(k == KF - 1))
                for k in range(KF):
                    nc.tensor.matmul(pb[:mw, :], lhsT=big1[:, k * S + ms:k * S + ms + mw],
                                     rhs=w1b_t[k][:], start=(k == 0), stop=(k == KF - 1))
                uab = sb.tile([P, H2], TBL_DT, tag="uab")
                nc.scalar.activation(out=uab[:mw, :H], in_=pa[:mw, :],
                                     func=mybir.ActivationFunctionType.Copy)
                nc.scalar.activation(out=uab[:mw, H:], in_=pb[:mw, :],
                                     func=mybir.ActivationFunctionType.Copy)
                nc.sync.dma_start(out=u_loc[ms:ms + mw, :], in_=uab[:mw, :])

            # ---------------- Phase B: AllGather u ------------------------
            if not single_core:
                nc.gpsimd.collective_compute(
                    "AllGather", mybir.AluOpType.bypass, replica_groups=groups,
                    ins=[u_loc[:]], outs=[U[:]])

            # ---------------- spmm tile emitter ---------------------------
            def spmm_tile(t, table, bias_t, relu, outT):
                """One dst tile: gather + aggregate + bias/act + transpose
                into feature-major outT [P, 4S]."""
                ts_, tw = mtile(t)
                nlo, nhi = int(cpw[t, 0]), int(cpw[t, 1])
                nch = nlo + nhi
                bl = blocks[t]
                nb = len(bl)
                mb0 = int(mblk_base[t])
                ph = ps_big.tile([P, H2], dt.float32, tag="ps_big")
                mt = mpool.tile([P, nb * P], TBL_DT, tag="mt")
                nc.sync.dma_start(out=mt[:], in_=Mt[:, mb0 * P:(mb0 + nb) * P])
                msg = msgp.tile([P, nch * H2], TBL_DT, tag="msg")
                for h, n_k, base, it in ((0, nlo, clo_base, idxl_t),
                                         (1, nhi, chi_base, idxh_t)):
                    o = int(base[t]) * 8
                    co = 0 if h == 0 else nlo
                    nc.gpsimd.dma_gather(
                        out_ap=msg[:, co * H2:(co + n_k) * H2].rearrange(
                            "p (n e) -> p n e", e=H2),
                        in_ap=table[:HALF, :] if h == 0 else table[HALF:, :],
                        idxs_ap=it[:, o:o + n_k * 8],
                        num_idxs=n_k * P, num_idxs_reg=n_k * P,
                        elem_size=H2)
                # bias opener (start=True zeroes the whole tile's psum)
                nc.tensor.matmul(ph[:, :], lhsT=ones_t[0:1, :],
                                 rhs=bias_t[0:1, :], start=True, stop=False,
                                 skip_group_check=True)
                for bi, (h, kind, idx, wwin) in enumerate(bl):
                    co = 0 if h == 0 else nlo
                    last = (bi == nb - 1)
                    if kind == "dr":
                        nc.tensor.matmul(
                            ph[64 * wwin:64 * wwin + 64, :],
                            lhsT=mt[:, bi * P:(bi + 1) * P].rearrange(
                                "p (i d) -> p i d", i=2),
                            rhs=msg[:, (co + 2 * idx) * H2:(co + 2 * idx + 2) * H2
                                    ].rearrange("p (i e) -> p i e", i=2),
                            start=False, stop=last, perf_mode=DR,
                            skip_group_check=True)
                    else:
                        nc.tensor.matmul(
                            ph[:, :],
                            lhsT=mt[:, bi * P:(bi + 1) * P],
                            rhs=msg[:, (co + idx) * H2:(co + idx + 1) * H2],
                            start=False, stop=last,
                            skip_group_check=True)
                hab = sb.tile([P, H2], dt.bfloat16, tag="hab")
                nc.scalar.activation(
                    out=hab[:tw, :], in_=ph[:tw, :],
                    func=(mybir.ActivationFunctionType.Relu if relu
                          else mybir.ActivationFunctionType.Copy))
                pt = ps_t.tile([P, H2], dt.bfloat16, tag="ps_t")
                for fc in range(2 * KH):
                    nc.tensor.transpose(out=pt[:, fc * P:fc * P + tw],
                                        in_=hab[:tw, fc * P:(fc + 1) * P],
                                        identity=ident[:tw, :tw])
                nc.scalar.activation(
                    out=outT[:, :].rearrange("p (f s) -> p f s", f=4)[:, :, ts_:ts_ + tw],
                    in_=pt[:, :].rearrange("p (f s) -> p f s", f=4)[:, :, :tw],
                    func=mybir.ActivationFunctionType.Copy)

            # -------- Phases C+D fused per tile: h = relu(spmm(U) + b1);
            # -------- v = h @ W2 ------------------------------------------
            for m in range(n_tiles):
                ms, mw = mtile(m)
                spmm_tile(m, U, b1_t, True, big0)
                pa = ps_d.tile([P, H], dt.float32, tag="ps_d")
                pb = ps_d.tile([P, H], dt.float32, tag="ps_d")
                for k in range(KH):
                    nc.tensor.matmul(pa[:mw, :], lhsT=big0[:, k * S + ms:k * S + ms + mw],
                                     rhs=w2a_t[k][:], start=(k == 0), stop=(k == KH - 1))
                for k in range(KH):
                    nc.tensor.matmul(pb[:mw, :],
                                     lhsT=big0[:, (KH + k) * S + ms:(KH + k) * S + ms + mw],
                                     rhs=w2b_t[k][:], start=(k == 0), stop=(k == KH - 1))
                vab = sb.tile([P, H2], TBL_DT, tag="vab")
                nc.scalar.activation(out=vab[:mw, :H], in_=pa[:mw, :],
                                     func=mybir.ActivationFunctionType.Copy)
                nc.scalar.activation(out=vab[:mw, H:], in_=pb[:mw, :],
                                     func=mybir.ActivationFunctionType.Copy)
                nc.sync.dma_start(out=v_loc[ms:ms + mw, :], in_=vab[:mw, :])

            # ---------------- Phase E: AllGather v ------------------------
            if not single_core:
                nc.gpsimd.collective_compute(
                    "AllGather", mybir.AluOpType.bypass, replica_groups=groups,
                    ins=[v_loc[:]], outs=[V[:]])

            # ---- Phases F+G+H fused per tile -----------------------------
            def softmax_z(py, zdst, mw, width):
                """zdst <- log_softmax(py) ; py is PSUM [P, width] f32 with
                the bias already accumulated (K=1 opener matmul)."""
                nmx = stat.tile([P, 1], dt.float32, tag="nmx")
                nc.vector.tensor_reduce(out=nmx[:mw, :], in_=py[:mw, :],
                                        axis=mybir.AxisListType.X,
                                        op=mybir.AluOpType.max, negate=True)
                ex = sb.tile([P, width], dt.float32, tag=f"ex{width}")
                sx = stat.tile([P, 1], dt.float32, tag="sx")
                nc.scalar.activation(out=ex[:mw, :], in_=py[:mw, :],
                                     func=mybir.ActivationFunctionType.Exp,
                                     bias=nmx[:mw, :], scale=1.0,
                                     accum_out=sx[:mw, :])
                lse = stat.tile([P, 1], dt.float32, tag="lse")
                nc.scalar.activation(out=lse[:mw, :], in_=sx[:mw, :],
                                     func=mybir.ActivationFunctionType.Ln)
                nc.vector.tensor_scalar(out=zdst, in0=py[:mw, :],
                                        scalar1=nmx[:mw, :], scalar2=lse[:mw, :],
                                        op0=mybir.AluOpType.add,
                                        op1=mybir.AluOpType.subtract)

            for m in range(n_tiles):
                ms, mw = mtile(m)
                # F: g = spmm(V) + b2 -> big1 feature-major
                spmm_tile(m, V, b2_t, False, big1)
                # G: z = log_softmax(g @ LW + Lb) -> big0 feature-major
                zab = sb.tile([P, H2], dt.bfloat16, tag="zab")
                for br, (lw_t, lb_t) in enumerate(
                        ((lwa_t, lba_t), (lwb_t, lbb_t))):
                    py = ps_d.tile([P, H], dt.float32, tag="ps_d")
                    nc.tensor.matmul(py[:, :], lhsT=ones_t[0:1, :],
                                     rhs=lb_t[0:1, :], start=True, stop=False,
                                     skip_group_check=True)
                    for k in range(KH):
                        nc.tensor.matmul(
                            py[:mw, :],
                            lhsT=big1[:, (2 * br + k) * S + ms:(2 * br + k) * S + ms + mw],
                            rhs=lw_t[k][:], start=False, stop=(k == KH - 1),
                            skip_group_check=True)
                    softmax_z(py, zab[:mw, br * H:(br + 1) * H], mw, H)
                pt = ps_t.tile([P, H2], dt.bfloat16, tag="ps_t")
                for fc in range(2 * KH):
                    nc.tensor.transpose(out=pt[:, fc * P:fc * P + mw],
                                        in_=zab[:mw, fc * P:(fc + 1) * P],
                                        identity=ident[:mw, :mw])
                nc.scalar.activation(
                    out=big0[:, :].rearrange("p (f s) -> p f s", f=4)[:, :, ms:ms + mw],
                    in_=pt[:, :].rearrange("p (f s) -> p f s", f=4)[:, :, :mw],
                    func=mybir.ActivationFunctionType.Copy)
                # H: out = log_softmax(z @ LWf + Lb)
                pf_full = ps_d.tile([P, H], dt.float32, name="pf_full", tag="ps_d")
                pf = pf_full[:, :C]
                nc.tensor.matmul(pf[:, :], lhsT=ones_t[0:1, :],
                                 rhs=lbf_t[0:1, :], start=True, stop=False,
                                 skip_group_check=True)
                for k in range(2 * KH):
                    nc.tensor.matmul(pf[:mw, :],
                                     lhsT=big0[:, k * S + ms:k * S + ms + mw],
                                     rhs=lwf_t[k][:], start=False,
                                     stop=(k == 2 * KH - 1),
                                     skip_group_check=True)
                ot = sb.tile([P, C], dt.float32, tag="ot")
                softmax_z(pf, ot[:mw, :], mw, C)
                nc.sync.dma_start(out=out_t[ms:ms + mw, :], in_=ot[:mw, :])

    import os
    if os.environ.get("NO_ACT_PIN"):
        nc.compile()
    else:
        with _pinned_act_tables():
            nc.compile()
    return nc


# ----------------------------------------------------------------------------
# Entry point
# ----------------------------------------------------------------------------

_CACHE = {}


def kernel(x0, x1, edge_src, edge_dst, edge_w,
           W1a, b1a, W2a, b2a, LWa, Lba,
           W1b, b1b, W2b, b2b, LWb, Lbb,
           LW, Lb):
    x0 = np.asarray(x0)
    x1 = np.asarray(x1)
    N, F0 = x0.shape
    H = np.asarray(W1a).shape[1]
    C = np.asarray(LW).shape[1]
    S = N // N_CORES

    key = (N, F0, H, C,
           hash(np.asarray(edge_src).tobytes()) ^ hash(np.asarray(edge_dst).tobytes()))
    if key not in _CACHE:
        plan, M_list, idxl_list, idxh_list = preprocess_edges(
            edge_src, edge_dst, edge_w, N, S)
        nc = build_nc(N, F0, H, C, S, plan)
        _CACHE[key] = (nc, M_list, idxl_list, idxh_list)
    nc, M_list, idxl_list, idxh_list = _CACHE[key]

    bf = lambda a: np.asarray(a, dtype=BF16)
    f32 = lambda a: np.asarray(a, dtype=np.float32)
    bcast = lambda v: np.broadcast_to(np.asarray(v, dtype=BF16)[None, :], (P, len(v))).copy()

    x0T = bf(x0).T
    x1T = bf(x1).T
    shared = {
        "W1a": bf(W1a), "W1b": bf(W1b), "W2a": bf(W2a), "W2b": bf(W2b),
        "LWa": bf(LWa), "LWb": bf(LWb), "LWf": bf(LW),
        "b1": bcast(np.concatenate([f32(b1a), f32(b1b)])),
        "b2": bcast(np.concatenate([f32(b2a), f32(b2b)])),
        "lba": bcast(f32(Lba)), "lbb": bcast(f32(Lbb)), "lbf": bcast(f32(Lb)),
    }
    in_maps = []
    for c in range(N_CORES):
        in_maps.append({
            **shared,
            "x0T": np.ascontiguousarray(x0T[:, c * S:(c + 1) * S]),
            "x1T": np.ascontiguousarray(x1T[:, c * S:(c + 1) * S]),
            "M": M_list[c], "IDXL": idxl_list[c], "IDXH": idxh_list[c],
        })
    res = run_bass_kernel_spmd(nc, in_maps, list(range(N_CORES)))
    return np.concatenate([res.results[c]["out"] for c in range(N_CORES)], axis=0)


# revision 13
# speedup vs baseline: 2.2251x; 1.1302x over previous
"""Bass/Trainium2 kernel for the 2-branch GCN (gnn_message_passing).

Computation (reference):
    per branch i in {a, b}:
        u_i = x_i @ W1_i                                  [N, H]
        h_i = relu(spmm(A, u_i) + b1_i)                   [N, H]
        v_i = h_i @ W2_i                                  [N, H]
        g_i = spmm(A, v_i) + b2_i                         [N, H]
        z_i = log_softmax(g_i @ LW_i + Lb_i)              [N, H]
    out = log_softmax(concat(z_a, z_b) @ LW + Lb)         [N, C]
where spmm(A, u)[d] = sum_{e: dst[e]=d} w[e] * u[src[e]].

Strategy (8 NeuronCores, node-sharded, fp8 message path):
  - Core c owns node rows [c*S, (c+1)*S), S = N/8.  Dense matmuls in bf16.
  - Activation tables U = allgather(x@W1), V = allgather(h@W2) stored fp8e4
    (concat a|b features -> 512B rows); both spmm layers gather rows of the
    concat table once per edge (512B descriptors, the 1x-latency minimum).
  - Edges grouped per (dst 128-tile, src half); chunk counts are padded to
    the max across the 8 cores so the compiled program is shared (SPMD).
  - Aggregation: one-hot matrices M (fp8, edge weight at the dst column)
    multiply gathered messages on the PE.  Chunk pairs whose 256 edges fall
    in one 64-dst window on ALL cores use a single DoubleRow fp8 matmul
    (0.5 cycles/row); mixed pairs emit two window-masked DoubleRow matmuls;
    a trailing odd chunk uses a plain [128,128] fp8 matmul.
  - Bias rides a K=1 matmul (ones x bias row) that also opens (start=True)
    each 64-row PSUM region; relu/cast psum->SBUF is one ACT op.
  - Feature-major activations live in two [128, 4S] SBUF tiles (h, g, z
    reuse the x0/x1 space); writeback per tile = 4 PE transposes into one
    PSUM bank + one 4-block strided ACT copy.
"""

import sys

if "/opt/trn_rl_repo" not in sys.path:
    sys.path.insert(0, "/opt/trn_rl_repo")

import numpy as np
import ml_dtypes

import concourse.bass as bass
import concourse.bacc as bacc
import concourse.mybir as mybir
import concourse.tile as tile
from concourse.tile import TileContext
from concourse.masks import make_identity
from concourse.bass_utils import run_bass_kernel_spmd

import contextlib
import concourse.bacc as _bacc_mod


@contextlib.contextmanager
def _pinned_act_tables():
    """During compile, make every activation-function table except the
    all-purpose one look empty so bacc's table-load inserter picks a single
    table for the whole program (one LoadActFuncSet instead of ~300)."""
    orig = _bacc_mod.get_activation_tables

    def pinned(arch):
        tabs = orig(arch)
        keep = "natural_log_exp_and_others"
        if keep in tabs:
            tabs = {k: (v if k == keep else set()) for k, v in tabs.items()}
        return tabs

    _bacc_mod.get_activation_tables = pinned
    try:
        yield
    finally:
        _bacc_mod.get_activation_tables = orig


BF16 = ml_dtypes.bfloat16
F8 = ml_dtypes.float8_e4m3
dt = mybir.dt
P = 128
N_CORES = 8
TBL_DT = dt.float8e4          # gather-table / message / M dtype


# ----------------------------------------------------------------------------
# Host-side edge preprocessing
# ----------------------------------------------------------------------------

def preprocess_edges(edge_src, edge_dst, edge_w, N, S):
    """Group edges per (dst 128-tile, src half), sorted by dst within each
    group.  Chunk = 128 gather slots; slot k*128+p holds sorted edge k*128+p.

    Emission plan (shared across cores):
      per (tile, half): for each pair of chunks j -> one DoubleRow matmul if
      the pair's edges lie in one 64-dst window on every core ("pure"), else
      two window-masked DoubleRow matmuls; a trailing odd chunk -> one plain
      [128,128] matmul.

    Returns (plan, M_list, idxl_list, idxh_list).
    """
    edge_src = np.asarray(edge_src).astype(np.int64)
    edge_dst = np.asarray(edge_dst).astype(np.int64)
    edge_w = np.asarray(edge_w, dtype=np.float32)
    n_tiles = (S + P - 1) // P
    HALF = N // 2

    per_core = []
    cnt = np.zeros((N_CORES, n_tiles, 2), dtype=np.int64)
    for c in range(N_CORES):
        sel = (edge_dst >= c * S) & (edge_dst < (c + 1) * S)
        dl = edge_dst[sel] - c * S
        sg = edge_src[sel]
        w = edge_w[sel]
        hi = (sg >= HALF).astype(np.int64)
        t = dl >> 7
        order = np.lexsort((dl, hi, t))
        dl, sg, w, hi, t = dl[order], sg[order], w[order], hi[order], t[order]
        gid = t * 2 + hi
        g = np.bincount(gid, minlength=2 * n_tiles)
        cnt[c] = g.reshape(n_tiles, 2)
        gstart = np.concatenate([[0], np.cumsum(g)])
        per_core.append((dl, sg, w, gid, gstart))

    cpw = np.maximum(1, (cnt.max(axis=0) + P - 1) // P)   # [n_tiles, 2]

    # ---- emission plan ----------------------------------------------------
    # blocks[t] = list of (h, kind, idx, w) in emission order; kind in
    # {"dr", "fat"}; idx = pair index j (dr) or chunk index k (fat);
    # w = 64-dst window (dr only; None for mixed covered via two entries).
    blocks = []
    nblk = np.zeros(n_tiles, dtype=np.int64)
    for t in range(n_tiles):
        bl = []
        for h in (0, 1):
            npair = int(cpw[t, h]) // 2
            odd = int(cpw[t, h]) % 2
            for j in range(npair):
                # pure if, on every core, all real edges of pair j fall in
                # one 64-window
                wset = set()
                for c in range(N_CORES):
                    dl, sg, w_, gid, gstart = per_core[c]
                    g0 = gstart[2 * t + h]
                    n = cnt[c, t, h]
                    a = min(256 * j, n)
                    b = min(256 * (j + 1), n)
                    if b > a:
                        dloc = dl[g0 + a:g0 + b] - 128 * t
                        if (dloc < 64).any():
                            wset.add(0)
                        if (dloc >= 64).any():
                            wset.add(1)
                if len(wset) <= 1:
                    bl.append((h, "dr", j, wset.pop() if wset else 0))
                else:
                    bl.append((h, "dr", j, 0))
                    bl.append((h, "dr", j, 1))
            if odd:
                bl.append((h, "fat", int(cpw[t, h]) - 1, None))
        blocks.append(bl)
        nblk[t] = len(bl)
    mblk_base = np.concatenate([[0], np.cumsum(nblk)])
    NBLK = int(mblk_base[-1])

    clo_base = np.concatenate([[0], np.cumsum(cpw[:, 0])])
    chi_base = np.concatenate([[0], np.cumsum(cpw[:, 1])])
    CLO, CHI = int(clo_base[-1]), int(chi_base[-1])

    # ---- per-core M / idx tensors ----------------------------------------
    M_list, idxl_list, idxh_list = [], [], []
    for c in range(N_CORES):
        dl, sg, w_, gid, gstart = per_core[c]
        M = np.zeros((P, NBLK * P), dtype=F8)
        idxl = np.zeros((P, CLO * 8), dtype=np.int16)
        idxh = np.zeros((P, CHI * 8), dtype=np.int16)
        for t in range(n_tiles):
            for bi, (h, kind, idx, wwin) in enumerate(blocks[t]):
                g0 = gstart[2 * t + h]
                n = int(cnt[c, t, h])
                col0 = (int(mblk_base[t]) + bi) * P
                if kind == "dr":
                    a = min(256 * idx, n)
                    b = min(256 * (idx + 1), n)
                    if b <= a:
                        continue
                    r = np.arange(a, b)
                    dloc = dl[g0 + a:g0 + b] - 128 * t
                    sel = (dloc >= 64) == (wwin == 1)
                    r, dloc = r[sel], dloc[sel]
                    i = (r - 256 * idx) >> 7
                    p = r & 127
                    M[p, col0 + i * 64 + (dloc - 64 * wwin)] = \
                        w_[g0 + r].astype(F8)
                else:  # fat
                    a = min(128 * idx, n)
                    b = min(128 * (idx + 1), n)
                    if b <= a:
                        continue
                    r = np.arange(a, b)
                    dloc = dl[g0 + a:g0 + b] - 128 * t
                    M[r & 127, col0 + dloc] = w_[g0 + r].astype(F8)
            # idx arrays: chunk k slot p -> sorted edge k*128+p (pad -> 0)
            for h, arr, base, off in ((0, idxl, clo_base, 0),
                                      (1, idxh, chi_base, HALF)):
                g0 = gstart[2 * t + h]
                n = int(cnt[c, t, h])
                nck = int(cpw[t, h])
                vals = np.zeros(nck * P, dtype=np.int16)
                vals[:n] = (sg[g0:g0 + n] - off).astype(np.int16)
                # wrapped layout: slot s -> row s%16 (replicated x8), col s//16
                cols = int(base[t]) * 8 + (np.arange(nck * P) >> 4)
                rows = np.arange(nck * P) & 15
                for g in range(8):
                    arr[16 * g + rows, cols] = vals
        M_list.append(M)
        idxl_list.append(idxl)
        idxh_list.append(idxh)

    plan = {
        "cpw": cpw, "blocks": blocks, "nblk": nblk, "mblk_base": mblk_base,
        "NBLK": NBLK, "clo_base": clo_base, "chi_base": chi_base,
        "CLO": CLO, "CHI": CHI, "n_tiles": n_tiles,
    }
    return plan, M_list, idxl_list, idxh_list


# ----------------------------------------------------------------------------
# Bass program
# ----------------------------------------------------------------------------

def build_nc(N, F0, H, C, S, plan, single_core=False):
    n_tiles = plan["n_tiles"]
    cpw = plan["cpw"]
    blocks = plan["blocks"]
    mblk_base = plan["mblk_base"]
    NBLK = plan["NBLK"]
    clo_base = plan["clo_base"]
    chi_base = plan["chi_base"]
    CLO, CHI = plan["CLO"], plan["CHI"]
    HALF = N // 2
    KF = F0 // P       # k-chunks of F0 (4)
    KH = H // P        # k-chunks of H (2)
    H2 = 2 * H
    DR = mybir.MatmulPerfMode.DoubleRow

    nc = bacc.Bacc("TRN2", num_devices=1 if single_core else N_CORES,
                   dynamic_dma_scratch_size=36864)

    # --- I/O ---
    x0T = nc.declare_dram_parameter("x0T", [F0, S], dt.bfloat16, isOutput=False)
    x1T = nc.declare_dram_parameter("x1T", [F0, S], dt.bfloat16, isOutput=False)
    W1a = nc.declare_dram_parameter("W1a", [F0, H], dt.bfloat16, isOutput=False)
    W1b = nc.declare_dram_parameter("W1b", [F0, H], dt.bfloat16, isOutput=False)
    W2a = nc.declare_dram_parameter("W2a", [H, H], dt.bfloat16, isOutput=False)
    W2b = nc.declare_dram_parameter("W2b", [H, H], dt.bfloat16, isOutput=False)
    LWa = nc.declare_dram_parameter("LWa", [H, H], dt.bfloat16, isOutput=False)
    LWb = nc.declare_dram_parameter("LWb", [H, H], dt.bfloat16, isOutput=False)
    LWf = nc.declare_dram_parameter("LWf", [H2, C], dt.bfloat16, isOutput=False)
    b1 = nc.declare_dram_parameter("b1", [P, H2], dt.bfloat16, isOutput=False)
    b2 = nc.declare_dram_parameter("b2", [P, H2], dt.bfloat16, isOutput=False)
    lba = nc.declare_dram_parameter("lba", [P, H], dt.bfloat16, isOutput=False)
    lbb = nc.declare_dram_parameter("lbb", [P, H], dt.bfloat16, isOutput=False)
    lbf = nc.declare_dram_parameter("lbf", [P, C], dt.bfloat16, isOutput=False)
    Mt = nc.declare_dram_parameter("M", [P, NBLK * P], TBL_DT, isOutput=False)
    IDXL = nc.declare_dram_parameter("IDXL", [P, CLO * 8], dt.int16, isOutput=False)
    IDXH = nc.declare_dram_parameter("IDXH", [P, CHI * 8], dt.int16, isOutput=False)
    out_t = nc.declare_dram_parameter("out", [S, C], dt.float32, isOutput=True)

    # --- internal DRAM ---
    u_loc = nc.dram_tensor("u_loc", [S, H2], TBL_DT)
    v_loc = nc.dram_tensor("v_loc", [S, H2], TBL_DT)
    if single_core:
        U = nc.declare_dram_parameter("Uin", [N, H2], TBL_DT, isOutput=False)
        V = nc.declare_dram_parameter("Vin", [N, H2], TBL_DT, isOutput=False)
    else:
        U = nc.dram_tensor("U", [N, H2], TBL_DT, addr_space="Shared")
        V = nc.dram_tensor("V", [N, H2], TBL_DT, addr_space="Shared")
    groups = [list(range(N_CORES))]

    with TileContext(nc, num_cores=N_CORES) as tc:
        ctx = contextlib.ExitStack()
        with ctx:
            perm = ctx.enter_context(tc.tile_pool(name="perm", bufs=1))
            big = ctx.enter_context(tc.tile_pool(name="big", bufs=1))
            mpool = ctx.enter_context(tc.tile_pool(name="mpool", bufs=3))
            msgp = ctx.enter_context(tc.tile_pool(name="msgp", bufs=3))
            sb = ctx.enter_context(tc.tile_pool(name="sb", bufs=3))
            stat = ctx.enter_context(tc.tile_pool(name="stat", bufs=4))
            ps_big = ctx.enter_context(tc.tile_pool(name="ps_big", bufs=3, space="PSUM"))
            ps_d = ctx.enter_context(tc.tile_pool(name="ps_d", bufs=3, space="PSUM"))
            ps_t = ctx.enter_context(tc.tile_pool(name="ps_t", bufs=2, space="PSUM"))

            # persistent small tiles
            ident = perm.tile([P, P], dt.bfloat16, tag="ident")
            make_identity(nc, ident[:])
            ones_t = perm.tile([P, P], dt.bfloat16, tag="ones")
            nc.vector.memset(ones_t[:], 1.0)
            w1a_t = [perm.tile([P, H], dt.bfloat16, name=f"w1a{k}", tag=f"w1a{k}") for k in range(KF)]
            w1b_t = [perm.tile([P, H], dt.bfloat16, name=f"w1b{k}", tag=f"w1b{k}") for k in range(KF)]
            w2a_t = [perm.tile([P, H], dt.bfloat16, name=f"w2a{k}", tag=f"w2a{k}") for k in range(KH)]
            w2b_t = [perm.tile([P, H], dt.bfloat16, name=f"w2b{k}", tag=f"w2b{k}") for k in range(KH)]
            lwa_t = [perm.tile([P, H], dt.bfloat16, name=f"lwa{k}", tag=f"lwa{k}") for k in range(KH)]
            lwb_t = [perm.tile([P, H], dt.bfloat16, name=f"lwb{k}", tag=f"lwb{k}") for k in range(KH)]
            lwf_t = [perm.tile([P, C], dt.bfloat16, name=f"lwf{k}", tag=f"lwf{k}") for k in range(2 * KH)]
            for k in range(KF):
                nc.sync.dma_start(out=w1a_t[k][:], in_=W1a[k * P:(k + 1) * P, :])
                nc.sync.dma_start(out=w1b_t[k][:], in_=W1b[k * P:(k + 1) * P, :])
            for k in range(KH):
                nc.sync.dma_start(out=w2a_t[k][:], in_=W2a[k * P:(k + 1) * P, :])
                nc.sync.dma_start(out=w2b_t[k][:], in_=W2b[k * P:(k + 1) * P, :])
                nc.sync.dma_start(out=lwa_t[k][:], in_=LWa[k * P:(k + 1) * P, :])
                nc.sync.dma_start(out=lwb_t[k][:], in_=LWb[k * P:(k + 1) * P, :])
            for k in range(2 * KH):
                nc.sync.dma_start(out=lwf_t[k][:], in_=LWf[k * P:(k + 1) * P, :])
            b1_t = perm.tile([P, H2], dt.bfloat16, tag="b1")
            b2_t = perm.tile([P, H2], dt.bfloat16, tag="b2")
            lba_t = perm.tile([P, H], dt.bfloat16, tag="lba")
            lbb_t = perm.tile([P, H], dt.bfloat16, tag="lbb")
            lbf_t = perm.tile([P, C], dt.bfloat16, tag="lbf")
            nc.sync.dma_start(out=b1_t[:], in_=b1[:])
            nc.sync.dma_start(out=b2_t[:], in_=b2[:])
            nc.sync.dma_start(out=lba_t[:], in_=lba[:])
            nc.sync.dma_start(out=lbb_t[:], in_=lbb[:])
            nc.sync.dma_start(out=lbf_t[:], in_=lbf[:])
            idxl_t = perm.tile([P, CLO * 8], dt.int16, tag="idxl")
            nc.sync.dma_start(out=idxl_t[:], in_=IDXL[:])
            idxh_t = perm.tile([P, CHI * 8], dt.int16, tag="idxh")
            nc.sync.dma_start(out=idxh_t[:], in_=IDXH[:])

            # two big feature-major tiles [P, 4S]; reused across phases:
            #   phase A in: big0 = x0T (4 k-chunks), big1 = x1T
            #   phase C out: big0 = hT (ha0 ha1 hb0 hb1 chunk-major)
            #   phase F out: big1 = gT
            #   phase G out: big0 = zT
            big0 = big.tile([P, 4 * S], dt.bfloat16, tag="big0")
            big1 = big.tile([P, 4 * S], dt.bfloat16, tag="big1")
            NQ = 4   # load x in column quarters so phase A starts early
            qb = [0] + [((q + 1) * S // NQ + P - 1) // P * P for q in range(NQ - 1)] + [S]
            for q in range(NQ):
                a, b = qb[q], qb[q + 1]
                for k in range(KF):
                    nc.sync.dma_start(out=big0[:, k * S + a:k * S + b],
                                      in_=x0T[k * P:(k + 1) * P, a:b])
                for k in range(KF):
                    nc.sync.dma_start(out=big1[:, k * S + a:k * S + b],
                                      in_=x1T[k * P:(k + 1) * P, a:b])

            def mtile(m):
                ms = m * P
                return ms, min(P, S - ms)

            # ---------------- Phase A: u = x @ W1 (both branches) ----------
            for m in range(n_tiles):
                ms, mw = mtile(m)
                pa = ps_d.tile([P, H], dt.float32, tag="ps_d")
                pb = ps_d.tile([P, H], dt.float32, tag="ps_d")
                for k in range(KF):
                    nc.tensor.matmul(pa[:mw, :], lhsT=big0[:, k * S + ms:k * S + ms + mw],
                                     rhs=w1a_t[k][:], start=(k == 0), stop=(k == KF - 1))
                for k in range(KF):
                    nc.tensor.matmul(pb[:mw, :], lhsT=big1[:, k * S + ms:k * S + ms + mw],
                                     rhs=w1b_t[k][:], start=(k == 0), stop=(k == KF - 1))
                uab = sb.tile([P, H2], TBL_DT, tag="uab")
                nc.scalar.activation(out=uab[:mw, :H], in_=pa[:mw, :],
                                     func=mybir.ActivationFunctionType.Copy)
                nc.scalar.activation(out=uab[:mw, H:], in_=pb[:mw, :],
                                     func=mybir.ActivationFunctionType.Copy)
                nc.sync.dma_start(out=u_loc[ms:ms + mw, :], in_=uab[:mw, :])

            # ---------------- Phase B: AllGather u ------------------------
            if not single_core:
                nc.gpsimd.collective_compute(
                    "AllGather", mybir.AluOpType.bypass, replica_groups=groups,
                    ins=[u_loc[:]], outs=[U[:]])

            # ---------------- spmm tile emitter ---------------------------
            def spmm_tile(t, table, bias_t, relu, outT):
                """One dst tile: gather + aggregate + bias/act + transpose
                into feature-major outT [P, 4S]."""
                ts_, tw = mtile(t)
                nlo, nhi = int(cpw[t, 0]), int(cpw[t, 1])
                nch = nlo + nhi
                bl = blocks[t]
                nb = len(bl)
                mb0 = int(mblk_base[t])
                ph = ps_big.tile([P, H2], dt.float32, tag="ps_big")
                mt = mpool.tile([P, nb * P], TBL_DT, tag="mt")
                nc.sync.dma_start(out=mt[:], in_=Mt[:, mb0 * P:(mb0 + nb) * P])
                msg = msgp.tile([P, nch * H2], TBL_DT, tag="msg")
                for h, n_k, base, it in ((0, nlo, clo_base, idxl_t),
                                         (1, nhi, chi_base, idxh_t)):
                    o = int(base[t]) * 8
                    co = 0 if h == 0 else nlo
                    nc.gpsimd.dma_gather(
                        out_ap=msg[:, co * H2:(co + n_k) * H2].rearrange(
                            "p (n e) -> p n e", e=H2),
                        in_ap=table[:HALF, :] if h == 0 else table[HALF:, :],
                        idxs_ap=it[:, o:o + n_k * 8],
                        num_idxs=n_k * P, num_idxs_reg=n_k * P,
                        elem_size=H2)
                # bias opener (start=True zeroes the whole tile's psum)
                nc.tensor.matmul(ph[:, :], lhsT=ones_t[0:1, :],
                                 rhs=bias_t[0:1, :], start=True, stop=False,
                                 skip_group_check=True)
                for bi, (h, kind, idx, wwin) in enumerate(bl):
                    co = 0 if h == 0 else nlo
                    last = (bi == nb - 1)
                    if kind == "dr":
                        nc.tensor.matmul(
                            ph[64 * wwin:64 * wwin + 64, :],
                            lhsT=mt[:, bi * P:(bi + 1) * P].rearrange(
                                "p (i d) -> p i d", i=2),
                            rhs=msg[:, (co + 2 * idx) * H2:(co + 2 * idx + 2) * H2
                                    ].rearrange("p (i e) -> p i e", i=2),
                            start=False, stop=last, perf_mode=DR,
                            skip_group_check=True)
                    else:
                        nc.tensor.matmul(
                            ph[:, :],
                            lhsT=mt[:, bi * P:(bi + 1) * P],
                            rhs=msg[:, (co + idx) * H2:(co + idx + 1) * H2],
                            start=False, stop=last,
                            skip_group_check=True)
                hab = sb.tile([P, H2], dt.bfloat16, tag="hab")
                nc.scalar.activation(
                    out=hab[:tw, :], in_=ph[:tw, :],
                    func=(mybir.ActivationFunctionType.Relu if relu
                          else mybir.ActivationFunctionType.Copy))
                pt = ps_t.tile([P, H2], dt.bfloat16, tag="ps_t")
                for fc in range(2 * KH):
                    nc.tensor.transpose(out=pt[:, fc * P:fc * P + tw],
                                        in_=hab[:tw, fc * P:(fc + 1) * P],
                                        identity=ident[:tw, :tw])
                nc.vector.tensor_scalar_add(
                    outT[:, :].rearrange("p (f s) -> p f s", f=4)[:, :, ts_:ts_ + tw],
                    pt[:, :].rearrange("p (f s) -> p f s", f=4)[:, :, :tw],
                    0.0)

            # -------- Phases C+D fused per tile: h = relu(spmm(U) + b1);
            # -------- v = h @ W2 ------------------------------------------
            def stage_D(m):
                ms, mw = mtile(m)
                pa = ps_d.tile([P, H], dt.float32, name="pa", tag="ps_d")
                pb = ps_d.tile([P, H], dt.float32, name="pb", tag="ps_d")
                for k in range(KH):
                    nc.tensor.matmul(pa[:mw, :], lhsT=big0[:, k * S + ms:k * S + ms + mw],
                                     rhs=w2a_t[k][:], start=(k == 0), stop=(k == KH - 1))
                for k in range(KH):
                    nc.tensor.matmul(pb[:mw, :],
                                     lhsT=big0[:, (KH + k) * S + ms:(KH + k) * S + ms + mw],
                                     rhs=w2b_t[k][:], start=(k == 0), stop=(k == KH - 1))
                vab = sb.tile([P, H2], TBL_DT, name="vab", tag="vab")
                nc.scalar.activation(out=vab[:mw, :H], in_=pa[:mw, :],
                                     func=mybir.ActivationFunctionType.Copy)
                nc.scalar.activation(out=vab[:mw, H:], in_=pb[:mw, :],
                                     func=mybir.ActivationFunctionType.Copy)
                nc.sync.dma_start(out=v_loc[ms:ms + mw, :], in_=vab[:mw, :])

            for m in range(n_tiles + 1):
                if m < n_tiles:
                    spmm_tile(m, U, b1_t, True, big0)
                if m >= 1:
                    stage_D(m - 1)

            # ---------------- Phase E: AllGather v ------------------------
            if not single_core:
                nc.gpsimd.collective_compute(
                    "AllGather", mybir.AluOpType.bypass, replica_groups=groups,
                    ins=[v_loc[:]], outs=[V[:]])

            # ---- Phases F+G+H fused per tile -----------------------------
            def softmax_z(py, zdst, mw, width):
                """zdst <- log_softmax(py) ; py is PSUM [P, width] f32 with
                the bias already accumulated (K=1 opener matmul)."""
                nmx = stat.tile([P, 1], dt.float32, tag="nmx")
                nc.vector.tensor_reduce(out=nmx[:mw, :], in_=py[:mw, :],
                                        axis=mybir.AxisListType.X,
                                        op=mybir.AluOpType.max, negate=True)
                ex = sb.tile([P, width], dt.float32, tag=f"ex{width}")
                sx = stat.tile([P, 1], dt.float32, tag="sx")
                nc.scalar.activation(out=ex[:mw, :], in_=py[:mw, :],
                                     func=mybir.ActivationFunctionType.Exp,
                                     bias=nmx[:mw, :], scale=1.0,
                                     accum_out=sx[:mw, :])
                lse = stat.tile([P, 1], dt.float32, tag="lse")
                nc.scalar.activation(out=lse[:mw, :], in_=sx[:mw, :],
                                     func=mybir.ActivationFunctionType.Ln)
                nc.vector.tensor_scalar(out=zdst, in0=py[:mw, :],
                                        scalar1=nmx[:mw, :], scalar2=lse[:mw, :],
                                        op0=mybir.AluOpType.add,
                                        op1=mybir.AluOpType.subtract)

            def stage_G(m):
                ms, mw = mtile(m)
                zab = sb.tile([P, H2], dt.bfloat16, name="zab", tag="zab")
                for br, (lw_t, lb_t) in enumerate(
                        ((lwa_t, lba_t), (lwb_t, lbb_t))):
                    py = ps_d.tile([P, H], dt.float32, name="py", tag="ps_d")
                    nc.tensor.matmul(py[:, :], lhsT=ones_t[0:1, :],
                                     rhs=lb_t[0:1, :], start=True, stop=False,
                                     skip_group_check=True)
                    for k in range(KH):
                        nc.tensor.matmul(
                            py[:mw, :],
                            lhsT=big1[:, (2 * br + k) * S + ms:(2 * br + k) * S + ms + mw],
                            rhs=lw_t[k][:], start=False, stop=(k == KH - 1),
                            skip_group_check=True)
                    softmax_z(py, zab[:mw, br * H:(br + 1) * H], mw, H)
                pt = ps_t.tile([P, H2], dt.bfloat16, name="pt", tag="ps_t")
                for fc in range(2 * KH):
                    nc.tensor.transpose(out=pt[:, fc * P:fc * P + mw],
                                        in_=zab[:mw, fc * P:(fc + 1) * P],
                                        identity=ident[:mw, :mw])
                nc.vector.tensor_scalar_add(
                    big0[:, :].rearrange("p (f s) -> p f s", f=4)[:, :, ms:ms + mw],
                    pt[:, :].rearrange("p (f s) -> p f s", f=4)[:, :, :mw],
                    0.0)

            def stage_H(m):
                ms, mw = mtile(m)
                pf_full = ps_d.tile([P, H], dt.float32, name="pf_full", tag="ps_d")
                pf = pf_full[:, :C]
                nc.tensor.matmul(pf[:, :], lhsT=ones_t[0:1, :],
                                 rhs=lbf_t[0:1, :], start=True, stop=False,
                                 skip_group_check=True)
                for k in range(2 * KH):
                    nc.tensor.matmul(pf[:mw, :],
                                     lhsT=big0[:, k * S + ms:k * S + ms + mw],
                                     rhs=lwf_t[k][:], start=False,
                                     stop=(k == 2 * KH - 1),
                                     skip_group_check=True)
                ot = sb.tile([P, C], dt.float32, name="ot", tag="ot")
                softmax_z(pf, ot[:mw, :], mw, C)
                nc.sync.dma_start(out=out_t[ms:ms + mw, :], in_=ot[:mw, :])

            for m in range(n_tiles + 2):
                if m < n_tiles:
                    spmm_tile(m, V, b2_t, False, big1)
                if 1 <= m <= n_tiles:
                    stage_G(m - 1)
                if m >= 2:
                    stage_H(m - 2)

    import os
    if os.environ.get("NO_ACT_PIN"):
        nc.compile()
    else:
        with _pinned_act_tables():
            nc.compile()
    return nc


# ----------------------------------------------------------------------------
# Entry point
# ----------------------------------------------------------------------------

_CACHE = {}


def kernel(x0, x1, edge_src, edge_dst, edge_w,
           W1a, b1a, W2a, b2a, LWa, Lba,
           W1b, b1b, W2b, b2b, LWb, Lbb,
           LW, Lb):
    x0 = np.asarray(x0)
    x1 = np.asarray(x1)
    N, F0 = x0.shape
    H = np.asarray(W1a).shape[1]
    C = np.asarray(LW).shape[1]
    S = N // N_CORES

    key = (N, F0, H, C,
           hash(np.asarray(edge_src).tobytes()) ^ hash(np.asarray(edge_dst).tobytes()))
    if key not in _CACHE:
        plan, M_list, idxl_list, idxh_list = preprocess_edges(
            edge_src, edge_dst, edge_w, N, S)
        nc = build_nc(N, F0, H, C, S, plan)
        _CACHE[key] = (nc, M_list, idxl_list, idxh_list)
    nc, M_list, idxl_list, idxh_list = _CACHE[key]

    bf = lambda a: np.asarray(a, dtype=BF16)
    f32 = lambda a: np.asarray(a, dtype=np.float32)
    bcast = lambda v: np.broadcast_to(np.asarray(v, dtype=BF16)[None, :], (P, len(v))).copy()

    x0T = bf(x0).T
    x1T = bf(x1).T
    shared = {
        "W1a": bf(W1a), "W1b": bf(W1b), "W2a": bf(W2a), "W2b": bf(W2b),
        "LWa": bf(LWa), "LWb": bf(LWb), "LWf": bf(LW),
        "b1": bcast(np.concatenate([f32(b1a), f32(b1b)])),
        "b2": bcast(np.concatenate([f32(b2a), f32(b2b)])),
        "lba": bcast(f32(Lba)), "lbb": bcast(f32(Lbb)), "lbf": bcast(f32(Lb)),
    }
    in_maps = []
    for c in range(N_CORES):
        in_maps.append({
            **shared,
            "x0T": np.ascontiguousarray(x0T[:, c * S:(c + 1) * S]),
            "x1T": np.ascontiguousarray(x1T[:, c * S:(c + 1) * S]),
            "M": M_list[c], "IDXL": idxl_list[c], "IDXH": idxh_list[c],
        })
    res = run_bass_kernel_spmd(nc, in_maps, list(range(N_CORES)))
    return np.concatenate([res.results[c]["out"] for c in range(N_CORES)], axis=0)
